# revision 1
# baseline (speedup 1.0000x reference)
"""Trainium2 Bass kernel for 2-layer GCN (GCNConv -> ReLU -> GCNConv).

v2 strategy — SBUF-resident fp16 tables + transpose-mode SBUF-source gathers
(the baseline's HBM dma_gather was HBM-latency bound at ~63ns/edge):

- Both layers reduce to: gather 16-wide rows t[src], segment-sum by dst
  (linear layers commute with the normalized aggregation).
- The 16-fp16 (32B) node records live in SBUF as [128 part, W windows] of
  256B chunks; chunk (tok, w) holds 8 records at positions s=0..7.
- A token (edge) gathers its source's 256B chunk via dma_gather(transpose=
  True, SBUF source): the chunk becomes a 128-partition fp16 column; the
  wanted record sits at partition slice [16s, 16s+16) where s = the record's
  chunk position ("class").  Chunk-mates land on other slices — never read.
- Host assigns each node TWO candidate classes and each edge picks one
  (power-of-two-choices), balancing per-(destination-group, class) slot
  counts K.  Grid columns per group g: [class s][slot k<K[g,s]][win j][dst p]
  so one strided DVE tensor_reduce per (group, class) segment-sums slot
  layers into slice s of a [128, 256] tile P.  P's 8 slices collapse via a
  PE matmul with a 0/1 selector (layer 2 fuses W2 into the selector).
- 3 SPMD launches: A (t1 = dinv*x@W1), B (layer-1 aggregate -> t2),
  C (layer-2 aggregate -> @W2+b2).  Host re-shards tables between launches.
"""
import os
import sys

sys.path.insert(0, "/opt/trn_rl_repo")

import numpy as np

import concourse.bass as bass
import concourse.mybir as mybir
import concourse.tile as tile
from concourse import bacc, bass_utils, library_config

N = 100000
E = 1600000
DIN, HID, DOUT = 256, 16, 64
NDEV = 8
NCLS = 8
GW = 1                      # windows per K-uniform group
F32 = mybir.dt.float32
F16 = mybir.dt.float16
I16 = mybir.dt.int16
NQUEUES = 1
GCHUNK = int(os.environ.get("GCN_GCHUNK", "2048"))
DMASCRATCH = int(os.environ.get("GCN_DMASCRATCH", "32768"))

LAST_EXEC_NS = []


# ----------------------------------------------------------------------------
# host-side graph planning
# ----------------------------------------------------------------------------

def _ragged_arange(lens):
    ends = np.cumsum(lens)
    total = int(ends[-1]) if len(lens) else 0
    out = np.arange(total, dtype=np.int64)
    out -= np.repeat(ends - lens, lens)
    return out


def _plan(edge_index, seed=12345):
    rng = np.random.default_rng(seed)
    src = np.asarray(edge_index[0], dtype=np.int64)
    dst = np.asarray(edge_index[1], dtype=np.int64)
    # self-loops are NOT tokens: the dst's own record is added in postproc
    all_src = src
    all_dst = dst
    T = len(all_src)
    indeg = np.bincount(dst, minlength=N).astype(np.int64) + 1  # GCN degree
    dinv_n = (1.0 / np.sqrt(indeg.astype(np.float64))).astype(np.float32)

    # rank deal: degree-sorted; i-th -> device i%8, window (i//8)//128
    order = np.argsort(-indeg, kind="stable")
    di = np.empty(N, np.int64)
    di[order] = np.arange(N)
    dev_n = di % NDEV
    w_n = (di // NDEV) // 128
    p_n = (di // NDEV) % 128
    nwin = int(w_n.max()) + 1
    ngrp = (nwin + GW - 1) // GW
    nwin_pad = ngrp * GW
    rank_n = (w_n * NDEV + dev_n) * 128 + p_n
    nloc = nwin_pad * 128
    npad = nloc * NDEV

    grp_n = w_n // GW

    # --- 2-choice class assignment ---
    s1 = rng.integers(0, NCLS, N)
    s2 = (s1 + 1 + rng.integers(0, NCLS - 1, N)) % NCLS
    ko = np.argsort(all_dst, kind="stable")
    t_dst = all_dst[ko]
    t_src = all_src[ko]
    starts = np.searchsorted(t_dst, np.arange(N + 1))
    pos = np.arange(T) - np.repeat(starts[:-1], np.diff(starts))
    cnt = np.zeros((N, NCLS), np.int16)
    cls_tok = np.zeros(T, np.int8)
    for k in range(int(np.diff(starts).max()) + 1):
        m = np.flatnonzero(pos == k)
        if len(m) == 0:
            break
        u, v = t_src[m], t_dst[m]
        c1, c2 = s1[u], s2[u]
        c = np.where(cnt[v, c1] <= cnt[v, c2], c1, c2)
        cls_tok[m] = c
        cnt[v, c] += 1

    def _getK(ct):
        K = np.zeros((ngrp, NCLS), np.int64)
        np.maximum.at(K, (grp_n[t_dst], ct.astype(np.int64)), cnt[t_dst, ct])
        return K

    K = _getK(cls_tok)
    best, best_cls = K.sum(), cls_tok.copy()
    tc1, tc2 = s1[t_src], s2[t_src]
    for _ in range(8):
        cur = cls_tok.astype(np.int64)
        alt = np.where(cur == tc1, tc2, tc1)
        v = t_dst
        b = grp_n[v]
        cand = np.flatnonzero((cnt[v, cur] == K[b, cur])
                              & (cnt[v, alt] + 1 < K[b, alt]) & (cur != alt))
        if len(cand) == 0:
            break
        key = v[cand] * NCLS + cur[cand]
        ks = np.argsort(key, kind="stable")
        kk = key[ks]
        first = np.concatenate([[True], kk[1:] != kk[:-1]])
        mv = cand[ks[first]]
        cv, av, vv = cur[mv], alt[mv], v[mv]
        cls_tok[mv] = av.astype(np.int8)
        np.subtract.at(cnt, (vv, cv), 1)
        np.add.at(cnt, (vv, av), 1)
        K = _getK(cls_tok)
        if K.sum() < best:
            best, best_cls = K.sum(), cls_tok.copy()
    if K.sum() != best:
        cls_tok = best_cls
        cnt = np.zeros((N, NCLS), np.int16)
        np.add.at(cnt, (t_dst, cls_tok.astype(np.int64)), 1)
        K = _getK(cls_tok)
    K = np.maximum(K, 1)

    # --- table slot allocation per class ---
    used = np.zeros((N, NCLS), bool)
    used[t_src, cls_tok.astype(np.int64)] = True
    gidx = np.zeros((N, NCLS), np.int32)
    slot_u, slot_s, slot_q = [], [], []
    nwt = 0
    for s in range(NCLS):
        us = np.flatnonzero(used[:, s])
        q = np.arange(len(us))
        gidx[us, s] = (q // 128) * 128 + (q % 128)
        slot_u.append(us)
        slot_s.append(np.full(len(us), s))
        slot_q.append(q)
        nwt = max(nwt, (len(us) + 127) // 128)
    slot_u = np.concatenate(slot_u)
    slot_s = np.concatenate(slot_s)
    slot_q = np.concatenate(slot_q)
    idx_pad = nwt * 128                    # zero window
    assert idx_pad + 127 < 32768

    # --- grid column layout (global K; identical on all devices) ---
    offs = np.concatenate([np.zeros((ngrp, 1), np.int64),
                           np.cumsum(K, axis=1)], axis=1) * (GW * 128)
    Cg = offs[:, -1]
    grpbase = np.concatenate([[0], np.cumsum(Cg)])
    T_dev = int(grpbase[-1])
    assert T_dev % 16 == 0

    # gather split per group: class boundary nearest the middle
    split = []
    for g in range(ngrp):
        sh = int(np.argmin(np.abs(offs[g, 1:-1] - Cg[g] / 2))) + 1
        split.append((sh, int(offs[g, sh])))

    # --- token -> column, idx arrays per device ---
    occ = np.empty(T, np.int64)
    key2 = t_dst * NCLS + cls_tok.astype(np.int64)
    k2o = np.argsort(key2, kind="stable")
    kk2 = key2[k2o]
    bnd = np.concatenate([[True], kk2[1:] != kk2[:-1]])
    gstarts = np.flatnonzero(bnd)
    glens = np.diff(np.concatenate([gstarts, [T]]))
    occ[k2o] = _ragged_arange(glens)

    v = t_dst
    g = grp_n[v]
    col = (grpbase[g] + offs[g, cls_tok.astype(np.int64)]
           + occ * (GW * 128) + (w_n[v] % GW) * 128 + p_n[v])
    tdev = dev_n[v]
    srcval = gidx[t_src, cls_tok.astype(np.int64)].astype(np.int16)
    idxw = np.empty((NDEV, 128, T_dev // 16), np.int16)
    for d in range(NDEV):
        m = tdev == d
        a = np.full(T_dev, idx_pad, np.int16)
        a[col[m]] = srcval[m]
        idxw[d] = np.tile(a.reshape(T_dev // 16, 16).T, (8, 1))

    # --- per-device aux arrays ---
    ridx = np.empty((NDEV, nloc), np.int64)
    for d in range(NDEV):
        gg = ((np.arange(nwin_pad) * NDEV + d)[:, None] * 128 + np.arange(128))
        ridx[d] = gg.reshape(-1)
    node_of_rank = np.full(npad, -1, np.int64)
    node_of_rank[rank_n] = np.arange(N)
    dinv_r = np.zeros(npad, np.float32)
    dinv_r[rank_n] = dinv_n

    dinva = np.empty((NDEV, 128, nwin_pad), np.float32)
    dinvw = np.empty((NDEV, 64, nloc), np.float16)
    for d in range(NDEV):
        dr = dinv_r[ridx[d]]
        dinva[d] = dr.reshape(nwin_pad, 128).T
        dinvw[d] = np.tile(dr[None, :], (64, 1)).astype(np.float16)

    return dict(
        nwin=nwin_pad, ngrp=ngrp, nloc=nloc, npad=npad, nwt=nwt,
        idx_pad=idx_pad, K=K, offs=offs, Cg=Cg, grpbase=grpbase,
        T_dev=T_dev, split=split, idxw=idxw, ridx=ridx,
        node_of_rank=node_of_rank, rank_n=rank_n, dinva=dinva, dinvw=dinvw,
        slot_u=slot_u, slot_s=slot_s, slot_q=slot_q,
    )


# ----------------------------------------------------------------------------
# device programs
# ----------------------------------------------------------------------------

def _build_A(plan):
    nwin, nloc = plan["nwin"], plan["nloc"]
    nc = bacc.Bacc("TRN2", target_bir_lowering=False, debug=False,
                   num_devices=NDEV)
    xT_d = nc.dram_tensor("xT", [DIN, nloc], F32, kind="ExternalInput").ap()
    dinva_d = nc.dram_tensor("dinva", [128, nwin], F32,
                             kind="ExternalInput").ap()
    w1_d = nc.dram_tensor("w1", [128, 2, HID], F32, kind="ExternalInput").ap()
    t1_d = nc.dram_tensor("t1", [nloc, HID], F16, kind="ExternalOutput").ap()

    with tile.TileContext(nc) as tc:
        with (
            tc.tile_pool(name="cst", bufs=1) as cst,
            tc.tile_pool(name="xp", bufs=3) as xp,
            tc.tile_pool(name="ps", bufs=2, space="PSUM") as psp,
            tc.tile_pool(name="stg", bufs=2) as stg,
        ):
            w1t = cst.tile([128, 2, HID], F32)
            nc.sync.dma_start(out=w1t[:], in_=w1_d[:])
            dat = cst.tile([128, nwin], F32)
            nc.sync.dma_start(out=dat[:], in_=dinva_d[:])
            ov = t1_d.rearrange("(w p) f -> p w f", p=128)
            for i0 in range(0, nwin, 8):
                nb = min(8, nwin - i0)
                xts = []
                for k in range(2):
                    xt = xp.tile([128, 8 * 128], F32, tag=f"xt{k}")
                    nc.sync.dma_start(
                        out=xt[:, :nb * 128],
                        in_=xT_d[k * 128:(k + 1) * 128,
                                 i0 * 128:(i0 + nb) * 128],
                    )
                    xts.append(xt)
                stage = stg.tile([128, 8, HID], F16)
                for ib in range(nb):
                    ps = psp.tile([128, HID], F32)
                    for k in range(2):
                        nc.tensor.matmul(
                            out=ps[:],
                            lhsT=xts[k][:, ib * 128:(ib + 1) * 128],
                            rhs=w1t[:, k, :],
                            start=(k == 0), stop=(k == 1),
                        )
                    nc.vector.tensor_scalar_mul(
                        out=stage[:, ib, :], in0=ps[:],
                        scalar1=dat[:, i0 + ib:i0 + ib + 1],
                    )
                nc.sync.dma_start(out=ov[:, i0:i0 + nb, :],
                                  in_=stage[:, :nb, :])
    nc.compile()
    return nc


def _sbuf_gather(nc, out_sl, tabt, idx_sl, ncols, q):
    nc.gpsimd.dma_gather(
        out_sl, tabt[:], idx_sl,
        num_idxs=ncols, num_idxs_reg=ncols, elem_size=128,
        transpose=True, single_packet=False, queue_num=q,
        sbuf_tokens_per_rank=128, sbuf_free_dim_per_rank=256,
        sbuf_free_dim_pad_per_rank=0, sbuf_byte_offset=0,
    )


def _reduce_cls(nc, Ps, vt, c0, Kgs):
    """Full-128-partition strided segment-sum of one class's token columns.
    Only partitions [16s, 16s+16) of the result are meaningful; the selector
    matmul extracts them (DVE cost is free-size driven, partitions are
    parallel lanes, and 16-partition slices at odd 16-offsets violate the
    32-alignment rule)."""
    sl = vt[:, 0, c0:c0 + GW * 128 * Kgs]
    rap = bass.AP(
        sl.tensor, sl.offset,
        [list(sl.ap[0]), [1, GW * 128], [GW * 128, Kgs]],
    )
    nc.vector.tensor_reduce(
        out=Ps[:], in_=rap,
        axis=mybir.AxisListType.X, op=mybir.AluOpType.add,
    )


def _build_BC(plan, layer):
    nwin, ngrp, nloc, nwt = (plan["nwin"], plan["ngrp"], plan["nloc"],
                             plan["nwt"])
    K, offs, Cg, grpbase, split, T_dev = (plan["K"], plan["offs"], plan["Cg"],
                                          plan["grpbase"], plan["split"],
                                          plan["T_dev"])
    WB = GW * 128
    cg_max = max(int(c) for c in Cg)

    nc = bacc.Bacc("TRN2", target_bir_lowering=False, debug=False,
                   num_devices=NDEV, num_swdge_queues=NQUEUES,
                   dynamic_dma_scratch_size=DMASCRATCH)
    tab_d = nc.dram_tensor("tab", [128, (nwt + 1) * 128], F16,
                           kind="ExternalInput").ap()
    idx_d = nc.dram_tensor("idx", [128, T_dev // 16], I16,
                           kind="ExternalInput").ap()
    dinvw_d = nc.dram_tensor("dinvw", [64, nloc], F16,
                             kind="ExternalInput").ap()
    if layer == 1:
        b_d = nc.dram_tensor("b1", [HID, 1], F32, kind="ExternalInput").ap()
        ts_d = nc.dram_tensor("tself", [HID, nloc], F16,
                              kind="ExternalInput").ap()
        sel_d = nc.dram_tensor("sel", [128, NCLS, HID], F32,
                               kind="ExternalInput").ap()
        id_d = nc.dram_tensor("ident", [HID, HID], F16,
                              kind="ExternalInput").ap()
        o_d = nc.dram_tensor("t2", [nloc, HID], F16,
                             kind="ExternalOutput").ap()
        ov = o_d.rearrange("(w p) f -> p w f", p=128)
    else:
        b_d = nc.dram_tensor("b2", [DOUT, 1], F32, kind="ExternalInput").ap()
        ts_d = nc.dram_tensor("wself", [DOUT, nloc], F16,
                              kind="ExternalInput").ap()
        sel_d = nc.dram_tensor("selw2", [128, NCLS, DOUT], F32,
                               kind="ExternalInput").ap()
        o_d = nc.dram_tensor("o2", [DOUT, nloc], F32,
                             kind="ExternalOutput").ap()
        ov = o_d

    with tile.TileContext(nc) as tc:
        with (
            tc.tile_pool(name="cst", bufs=1) as cst,
            tc.tile_pool(name="ip", bufs=3) as ip,
            tc.tile_pool(name="vp", bufs=2) as vp,
            tc.tile_pool(name="pp", bufs=3) as pp,
            tc.tile_pool(name="sm", bufs=3) as sm,
            tc.tile_pool(name="pY", bufs=2, space="PSUM") as pY,
            tc.tile_pool(name="pT", bufs=2, space="PSUM") as pT,
            tc.tile_pool(name="stg", bufs=2) as stg,
        ):
            nc.gpsimd.load_library(library_config.mlp)
            tabt = cst.tile([128, (nwt + 1) * 128], F16)
            nc.sync.dma_start(out=tabt[:], in_=tab_d[:])
            dvw = cst.tile([64, nloc], F16)
            nc.sync.dma_start(out=dvw[:], in_=dinvw_d[:])
            nb_ = HID if layer == 1 else DOUT
            bt = cst.tile([nb_, 1], F32)
            nc.sync.dma_start(out=bt[:], in_=b_d[:])
            tsl = cst.tile([nb_, nloc], F16)
            nc.sync.dma_start(out=tsl[:], in_=ts_d[:])
            selt = cst.tile([128, NCLS, nb_], F32)
            nc.sync.dma_start(out=selt[:], in_=sel_d[:])
            if layer == 1:
                idt = cst.tile([HID, HID], F16)
                nc.sync.dma_start(out=idt[:], in_=id_d[:])

            it_max = cg_max // 16
            for g in range(ngrp):
                cg = int(Cg[g])
                t0 = int(grpbase[g])
                it = ip.tile([128, it_max], I16, tag="idx")
                nc.sync.dma_start(out=it[:, :cg // 16],
                                  in_=idx_d[:, t0 // 16:(t0 + cg) // 16])
                va = vp.tile([128, 1, cg_max], F16, tag="va")
                # <=4096-idx gathers fit the ~256-desc/engine SWDGE ring, so
                # gather N+1's descriptor generation overlaps gather N's drain
                nch = (cg + GCHUNK - 1) // GCHUNK
                bnds = [(cg * i // nch) // 128 * 128 for i in range(nch + 1)]
                bnds[-1] = cg
                for c0, c1 in zip(bnds[:-1], bnds[1:]):
                    _sbuf_gather(nc, va[:, :, c0:c1], tabt,
                                 it[:, c0 // 16:c1 // 16], c1 - c0, 0)
                Y = pY.tile([nb_, WB], F32)
                for s in range(NCLS):
                    Kgs = int(K[g, s])
                    Ps = pp.tile([128, WB], F32, tag=f"P{s}")
                    _reduce_cls(nc, Ps, va, int(offs[g, s]), Kgs)
                    nc.tensor.matmul(out=Y[:], lhsT=selt[:, s, :], rhs=Ps[:],
                                     start=(s == 0), stop=(s == NCLS - 1))
                dsl = dvw[0:nb_, g * WB:(g + 1) * WB]
                ya = sm.tile([nb_, WB], F32, tag="ya")
                nc.vector.scalar_tensor_tensor(
                    out=ya[:], in0=Y[:], scalar=1.0,
                    in1=tsl[:, g * WB:(g + 1) * WB],
                    op0=mybir.AluOpType.mult, op1=mybir.AluOpType.add,
                )
                if layer == 1:
                    yd = sm.tile([HID, WB], F32, tag="yd")
                    nc.vector.scalar_tensor_tensor(
                        out=yd[:], in0=ya[:], scalar=1.0, in1=dsl,
                        op0=mybir.AluOpType.mult, op1=mybir.AluOpType.mult,
                    )
                    r = sm.tile([HID, WB], F32, tag="r")
                    nc.scalar.activation(
                        out=r[:], in_=yd[:],
                        func=mybir.ActivationFunctionType.Relu,
                        bias=bt[:, 0:1],
                    )
                    t2c = sm.tile([HID, WB], F16, tag="t2c")
                    nc.vector.scalar_tensor_tensor(
                        out=t2c[:], in0=r[:], scalar=1.0, in1=dsl,
                        op0=mybir.AluOpType.mult, op1=mybir.AluOpType.mult,
                    )
                    stage = stg.tile([128, GW, HID], F16)
                    for j in range(GW):
                        tp = pT.tile([128, HID], F16)
                        nc.tensor.transpose(
                            out=tp[:], in_=t2c[:, j * 128:(j + 1) * 128],
                            identity=idt[:])
                        nc.vector.tensor_copy(out=stage[:, j, :], in_=tp[:])
                    nc.sync.dma_start(out=ov[:, g * GW:(g + 1) * GW, :],
                                      in_=stage[:])
                else:
                    od = sm.tile([DOUT, WB], F32, tag="od")
                    nc.vector.scalar_tensor_tensor(
                        out=od[:], in0=ya[:], scalar=1.0, in1=dsl,
                        op0=mybir.AluOpType.mult, op1=mybir.AluOpType.mult,
                    )
                    stage = stg.tile([DOUT, WB], F32)
                    nc.vector.tensor_scalar_add(
                        out=stage[:], in0=od[:], scalar1=bt[:, 0:1],
                    )
                    nc.sync.dma_start(out=ov[:, g * WB:(g + 1) * WB],
                                      in_=stage[:])
    nc.compile()
    return nc


# ----------------------------------------------------------------------------
# driver
# ----------------------------------------------------------------------------

_PROG_CACHE = {}


def _run(nc, in_maps):
    trace = os.environ.get("GCN_TRACE", "0") == "1"
    res = bass_utils.run_bass_kernel_spmd(
        nc, in_maps, core_ids=list(range(NDEV)), trace=trace)
    if res.exec_time_ns is not None:
        LAST_EXEC_NS.append(int(res.exec_time_ns))
    return res.results


def _build_table(plan, t_node):
    nwt = plan["nwt"]
    tab = np.zeros((128, nwt + 1, NCLS, HID), np.float16)
    q, s, u = plan["slot_q"], plan["slot_s"], plan["slot_u"]
    tab[q % 128, q // 128, s] = t_node[u]
    return np.ascontiguousarray(tab.reshape(128, (nwt + 1) * 128))


def kernel(x, edge_index, W1, b1, W2, b2):
    LAST_EXEC_NS.clear()
    x = np.asarray(x, np.float32)
    W1 = np.asarray(W1, np.float32)
    b1 = np.asarray(b1, np.float32)
    W2 = np.asarray(W2, np.float32)
    b2 = np.asarray(b2, np.float32)

    ei = np.asarray(edge_index)
    plan = min((_plan(ei, seed=s) for s in (12345, 7, 99)),
               key=lambda p: p["T_dev"])
    key = ("v4", plan["nwin"], plan["T_dev"], int(plan["K"].sum()))
    if key not in _PROG_CACHE:
        _PROG_CACHE.clear()
        _PROG_CACHE[key] = (_build_A(plan), _build_BC(plan, 1),
                            _build_BC(plan, 2))
    ncA, ncB, ncC = _PROG_CACHE[key]

    ridx, nor = plan["ridx"], plan["node_of_rank"]
    npad, nloc = plan["npad"], plan["nloc"]

    xfull = np.zeros((npad, DIN), np.float32)
    xfull[plan["rank_n"]] = x
    w1r = np.ascontiguousarray(
        W1.reshape(2, 128, HID).transpose(1, 0, 2))
    inA = [{"xT": np.ascontiguousarray(xfull[ridx[d]].T),
            "dinva": plan["dinva"][d], "w1": w1r} for d in range(NDEV)]
    resA = _run(ncA, inA)
    t1n = np.zeros((N, HID), np.float16)
    for d in range(NDEV):
        m = nor[ridx[d]] >= 0
        t1n[nor[ridx[d]][m]] = resA[d]["t1"][m]

    sel = np.zeros((128, NCLS, HID), np.float32)
    for s in range(NCLS):
        sel[16 * s + np.arange(HID), s, np.arange(HID)] = 1.0
    def _self_arr(tn):
        out = []
        for d in range(NDEV):
            a = np.zeros((nloc, tn.shape[1]), np.float16)
            m = nor[ridx[d]] >= 0
            a[m] = tn[nor[ridx[d]][m]]
            out.append(np.ascontiguousarray(a.T))
        return out

    ts1 = _self_arr(t1n)
    inB = [{"tab": _build_table(plan, t1n), "idx": plan["idxw"][d],
            "dinvw": plan["dinvw"][d], "b1": b1[:, None].astype(np.float32),
            "tself": ts1[d], "sel": sel,
            "ident": np.eye(HID, dtype=np.float16)}
           for d in range(NDEV)]
    resB = _run(ncB, inB)
    t2n = np.zeros((N, HID), np.float16)
    for d in range(NDEV):
        m = nor[ridx[d]] >= 0
        t2n[nor[ridx[d]][m]] = resB[d]["t2"][m]

    selw2 = np.zeros((128, NCLS, DOUT), np.float32)
    for s in range(NCLS):
        selw2[16 * s + np.arange(HID), s, :] = W2
    ws = _self_arr((t2n.astype(np.float32) @ W2).astype(np.float16))
    inC = [{"tab": _build_table(plan, t2n), "idx": plan["idxw"][d],
            "dinvw": plan["dinvw"][d], "b2": b2[:, None].astype(np.float32),
            "wself": ws[d], "selw2": selw2} for d in range(NDEV)]
    resC = _run(ncC, inC)
    out = np.zeros((N, DOUT), np.float32)
    for d in range(NDEV):
        m = nor[ridx[d]] >= 0
        out[nor[ridx[d]][m]] = resC[d]["o2"].T[m]
    return out



# revision 8
# speedup vs baseline: 1.1131x; 1.1131x over previous
"""Trainium2 Bass kernel for 2-layer GCN (GCNConv -> ReLU -> GCNConv).

v2 strategy — SBUF-resident fp16 tables + transpose-mode SBUF-source gathers
(the baseline's HBM dma_gather was HBM-latency bound at ~63ns/edge):

- Both layers reduce to: gather 16-wide rows t[src], segment-sum by dst
  (linear layers commute with the normalized aggregation).
- The 16-fp16 (32B) node records live in SBUF as [128 part, W windows] of
  256B chunks; chunk (tok, w) holds 8 records at positions s=0..7.
- A token (edge) gathers its source's 256B chunk via dma_gather(transpose=
  True, SBUF source): the chunk becomes a 128-partition fp16 column; the
  wanted record sits at partition slice [16s, 16s+16) where s = the record's
  chunk position ("class").  Chunk-mates land on other slices — never read.
- Host assigns each node TWO candidate classes and each edge picks one
  (power-of-two-choices), balancing per-(destination-group, class) slot
  counts K.  Grid columns per group g: [class s][slot k<K[g,s]][win j][dst p]
  so one strided DVE tensor_reduce per (group, class) segment-sums slot
  layers into slice s of a [128, 256] tile P.  P's 8 slices collapse via a
  PE matmul with a 0/1 selector (layer 2 fuses W2 into the selector).
- 3 SPMD launches: A (t1 = dinv*x@W1), B (layer-1 aggregate -> t2),
  C (layer-2 aggregate -> @W2+b2).  Host re-shards tables between launches.
"""
import os
import sys

sys.path.insert(0, "/opt/trn_rl_repo")

import numpy as np

import concourse.bass as bass
import concourse.mybir as mybir
import concourse.tile as tile
from concourse import bacc, bass_utils, library_config

N = 100000
E = 1600000
DIN, HID, DOUT = 256, 16, 64
NDEV = 8
NCLS = 8
GW = 1                      # windows per K-uniform group
F32 = mybir.dt.float32
F16 = mybir.dt.float16
I16 = mybir.dt.int16
NQUEUES = int(os.environ.get("GCN_NQUEUES", "1"))
GCHUNK = int(os.environ.get("GCN_GCHUNK", "2048"))
DMASCRATCH = int(os.environ.get("GCN_DMASCRATCH", "32768"))
N3 = int(os.environ.get("GCN_N3", "62000"))
MAXSLOTS = 32640              # 255 windows/class keeps idx_pad in int16

LAST_EXEC_NS = []


# ----------------------------------------------------------------------------
# host-side graph planning
# ----------------------------------------------------------------------------

def _ragged_arange(lens):
    ends = np.cumsum(lens)
    total = int(ends[-1]) if len(lens) else 0
    out = np.arange(total, dtype=np.int64)
    out -= np.repeat(ends - lens, lens)
    return out


def _plan(edge_index, seed=12345):
    rng = np.random.default_rng(seed)
    src = np.asarray(edge_index[0], dtype=np.int64)
    dst = np.asarray(edge_index[1], dtype=np.int64)
    # self-loops are NOT tokens: the dst's own record is added in postproc
    all_src = src
    all_dst = dst
    T = len(all_src)
    indeg = np.bincount(dst, minlength=N).astype(np.int64) + 1  # GCN degree
    dinv_n = (1.0 / np.sqrt(indeg.astype(np.float64))).astype(np.float32)

    # rank deal: degree-sorted; i-th -> device i%8, window (i//8)//128
    order = np.argsort(-indeg, kind="stable")
    di = np.empty(N, np.int64)
    di[order] = np.arange(N)
    dev_n = di % NDEV
    w_n = (di // NDEV) // 128
    p_n = (di // NDEV) % 128
    nwin = int(w_n.max()) + 1
    ngrp = (nwin + GW - 1) // GW
    nwin_pad = ngrp * GW
    rank_n = (w_n * NDEV + dev_n) * 128 + p_n
    nloc = nwin_pad * 128
    npad = nloc * NDEV

    grp_n = w_n // GW

    # --- class assignment: mixed-R candidates + balanced greedy + repair ---
    ko = np.argsort(all_dst, kind="stable")
    t_dst = all_dst[ko]
    t_src = all_src[ko]
    tg = grp_n[t_dst]

    def _make_cand(n3):
        s1 = rng.integers(0, NCLS, N)
        s2 = (s1 + 1 + rng.integers(0, NCLS - 1, N)) % NCLS
        cand = np.stack([s1, s2], 1)
        if n3 > 0:
            odeg = np.bincount(all_src, minlength=N)
            top = np.argsort(-odeg, kind="stable")[:n3]
            s3 = rng.integers(0, NCLS, n3)
            bad = (s3 == s1[top]) | (s3 == s2[top])
            while bad.any():
                s3[bad] = rng.integers(0, NCLS, bad.sum())
                bad = (s3 == s1[top]) | (s3 == s2[top])
            c3 = np.full(N, -1, np.int64)
            c3[top] = s3
            cand = np.concatenate([cand, c3[:, None]], 1)
        return cand

    starts = np.searchsorted(t_dst, np.arange(N + 1))
    pos = np.arange(T) - np.repeat(starts[:-1], np.diff(starts))

    def _greedy(cand):
        cnt = np.zeros((N, NCLS), np.int16)
        cls_tok = np.zeros(T, np.int8)
        for k in range(int(pos.max()) + 1):
            m = np.flatnonzero(pos == k)
            if len(m) == 0:
                break
            u, v = t_src[m], t_dst[m]
            cc = cand[u]
            counts = np.where(cc >= 0, cnt[v[:, None], np.maximum(cc, 0)], 127)
            best = np.argmin(counts, axis=1)
            c = cc[np.arange(len(m)), best]
            cls_tok[m] = c
            cnt[v, c] += 1
        return cls_tok, cnt

    def _repair(cls_tok, cnt, cand, iters=200):
        R = cand.shape[1]
        cls_tok = cls_tok.astype(np.int64)
        tc = cand[t_src]
        last = -1
        for it in range(iters):
            K = np.zeros((ngrp, NCLS), np.int64)
            np.maximum.at(K, (tg, cls_tok), cnt[t_dst, cls_tok])
            K = np.maximum(K, 1)
            if it % 20 == 0:
                s = int(K.sum())
                if s == last:
                    break
                last = s
            cur = cls_tok
            crit = cnt[t_dst, cur] == K[tg, cur]
            alt_ok = np.full(T, -1, np.int64)
            for r in rng.permutation(R):
                a = tc[:, r]
                a0 = np.maximum(a, 0)
                ok = (crit & (a >= 0) & (a != cur) & (alt_ok < 0)
                      & (cnt[t_dst, a0] + 1 < K[tg, a0]))
                alt_ok[ok] = a[ok]
            mv = np.flatnonzero(alt_ok >= 0)
            if len(mv) == 0:
                break
            ks = np.argsort(t_dst[mv], kind="stable")
            kk = t_dst[mv][ks]
            first = np.concatenate([[True], kk[1:] != kk[:-1]])
            mv = mv[ks[first]]
            cv, av, vv = cur[mv].copy(), alt_ok[mv], t_dst[mv]
            cls_tok[mv] = av
            np.subtract.at(cnt, (vv, cv), 1)
            np.add.at(cnt, (vv, av), 1)
        K = np.zeros((ngrp, NCLS), np.int64)
        np.maximum.at(K, (tg, cls_tok), cnt[t_dst, cls_tok])
        return cls_tok.astype(np.int8), cnt, np.maximum(K, 1)

    n3 = N3
    while True:
        cand = _make_cand(n3)
        cls_tok, cnt = _greedy(cand)
        cls_tok, cnt, K = _repair(cls_tok, cnt, cand)
        used_chk = np.zeros((N, NCLS), bool)
        used_chk[t_src, cls_tok.astype(np.int64)] = True
        if used_chk.sum(0).max() <= MAXSLOTS or n3 == 0:
            break
        n3 = max(0, n3 - 15000)

    # --- table slot allocation per class ---
    used = np.zeros((N, NCLS), bool)
    used[t_src, cls_tok.astype(np.int64)] = True
    gidx = np.zeros((N, NCLS), np.int32)
    slot_u, slot_s, slot_q = [], [], []
    nwt = 0
    for s in range(NCLS):
        us = np.flatnonzero(used[:, s])
        q = np.arange(len(us))
        gidx[us, s] = (q // 128) * 128 + (q % 128)
        slot_u.append(us)
        slot_s.append(np.full(len(us), s))
        slot_q.append(q)
        nwt = max(nwt, (len(us) + 127) // 128)
    slot_u = np.concatenate(slot_u)
    slot_s = np.concatenate(slot_s)
    slot_q = np.concatenate(slot_q)
    idx_pad = nwt * 128                    # zero window
    assert idx_pad + 127 < 32768

    # --- grid column layout (global K; identical on all devices) ---
    offs = np.concatenate([np.zeros((ngrp, 1), np.int64),
                           np.cumsum(K, axis=1)], axis=1) * (GW * 128)
    Cg = offs[:, -1]
    grpbase = np.concatenate([[0], np.cumsum(Cg)])
    T_dev = int(grpbase[-1])
    assert T_dev % 16 == 0

    # gather split per group: class boundary nearest the middle
    split = []
    for g in range(ngrp):
        sh = int(np.argmin(np.abs(offs[g, 1:-1] - Cg[g] / 2))) + 1
        split.append((sh, int(offs[g, sh])))

    # --- token -> column, idx arrays per device ---
    occ = np.empty(T, np.int64)
    key2 = t_dst * NCLS + cls_tok.astype(np.int64)
    k2o = np.argsort(key2, kind="stable")
    kk2 = key2[k2o]
    bnd = np.concatenate([[True], kk2[1:] != kk2[:-1]])
    gstarts = np.flatnonzero(bnd)
    glens = np.diff(np.concatenate([gstarts, [T]]))
    occ[k2o] = _ragged_arange(glens)

    v = t_dst
    g = grp_n[v]
    col = (grpbase[g] + offs[g, cls_tok.astype(np.int64)]
           + occ * (GW * 128) + (w_n[v] % GW) * 128 + p_n[v])
    tdev = dev_n[v]
    srcval = gidx[t_src, cls_tok.astype(np.int64)].astype(np.int16)
    idxw = np.empty((NDEV, 128, T_dev // 16), np.int16)
    for d in range(NDEV):
        m = tdev == d
        a = np.full(T_dev, idx_pad, np.int16)
        a[col[m]] = srcval[m]
        idxw[d] = np.tile(a.reshape(T_dev // 16, 16).T, (8, 1))

    # --- per-device aux arrays ---
    ridx = np.empty((NDEV, nloc), np.int64)
    for d in range(NDEV):
        gg = ((np.arange(nwin_pad) * NDEV + d)[:, None] * 128 + np.arange(128))
        ridx[d] = gg.reshape(-1)
    node_of_rank = np.full(npad, -1, np.int64)
    node_of_rank[rank_n] = np.arange(N)
    dinv_r = np.zeros(npad, np.float32)
    dinv_r[rank_n] = dinv_n

    dinva = np.empty((NDEV, 128, nwin_pad), np.float32)
    dinvw = np.empty((NDEV, 64, nloc), np.float16)
    for d in range(NDEV):
        dr = dinv_r[ridx[d]]
        dinva[d] = dr.reshape(nwin_pad, 128).T
        dinvw[d] = np.tile(dr[None, :], (64, 1)).astype(np.float16)

    return dict(
        nwin=nwin_pad, ngrp=ngrp, nloc=nloc, npad=npad, nwt=nwt,
        idx_pad=idx_pad, K=K, offs=offs, Cg=Cg, grpbase=grpbase,
        T_dev=T_dev, split=split, idxw=idxw, ridx=ridx,
        node_of_rank=node_of_rank, rank_n=rank_n, dinva=dinva, dinvw=dinvw,
        slot_u=slot_u, slot_s=slot_s, slot_q=slot_q,
    )


# ----------------------------------------------------------------------------
# device programs
# ----------------------------------------------------------------------------

def _build_A(plan):
    nwin, nloc = plan["nwin"], plan["nloc"]
    nc = bacc.Bacc("TRN2", target_bir_lowering=False, debug=False,
                   num_devices=NDEV)
    xT_d = nc.dram_tensor("xT", [DIN, nloc], F32, kind="ExternalInput").ap()
    dinva_d = nc.dram_tensor("dinva", [128, nwin], F32,
                             kind="ExternalInput").ap()
    w1_d = nc.dram_tensor("w1", [128, 2, HID], F32, kind="ExternalInput").ap()
    t1_d = nc.dram_tensor("t1", [nloc, HID], F16, kind="ExternalOutput").ap()

    with tile.TileContext(nc) as tc:
        with (
            tc.tile_pool(name="cst", bufs=1) as cst,
            tc.tile_pool(name="xp", bufs=3) as xp,
            tc.tile_pool(name="ps", bufs=2, space="PSUM") as psp,
            tc.tile_pool(name="stg", bufs=2) as stg,
        ):
            w1t = cst.tile([128, 2, HID], F32)
            nc.sync.dma_start(out=w1t[:], in_=w1_d[:])
            dat = cst.tile([128, nwin], F32)
            nc.sync.dma_start(out=dat[:], in_=dinva_d[:])
            ov = t1_d.rearrange("(w p) f -> p w f", p=128)
            for i0 in range(0, nwin, 8):
                nb = min(8, nwin - i0)
                xts = []
                for k in range(2):
                    xt = xp.tile([128, 8 * 128], F32, tag=f"xt{k}")
                    nc.sync.dma_start(
                        out=xt[:, :nb * 128],
                        in_=xT_d[k * 128:(k + 1) * 128,
                                 i0 * 128:(i0 + nb) * 128],
                    )
                    xts.append(xt)
                stage = stg.tile([128, 8, HID], F16)
                for ib in range(nb):
                    ps = psp.tile([128, HID], F32)
                    for k in range(2):
                        nc.tensor.matmul(
                            out=ps[:],
                            lhsT=xts[k][:, ib * 128:(ib + 1) * 128],
                            rhs=w1t[:, k, :],
                            start=(k == 0), stop=(k == 1),
                        )
                    nc.vector.tensor_scalar_mul(
                        out=stage[:, ib, :], in0=ps[:],
                        scalar1=dat[:, i0 + ib:i0 + ib + 1],
                    )
                nc.sync.dma_start(out=ov[:, i0:i0 + nb, :],
                                  in_=stage[:, :nb, :])
    nc.compile()
    return nc


def _sbuf_gather(nc, out_sl, tabt, idx_sl, ncols, q):
    nc.gpsimd.dma_gather(
        out_sl, tabt[:], idx_sl,
        num_idxs=ncols, num_idxs_reg=ncols, elem_size=128,
        transpose=True, single_packet=False, queue_num=q,
        sbuf_tokens_per_rank=128, sbuf_free_dim_per_rank=256,
        sbuf_free_dim_pad_per_rank=0, sbuf_byte_offset=0,
    )


def _reduce_cls(nc, Ps, vt, c0, Kgs):
    """Full-128-partition strided segment-sum of one class's token columns.
    Only partitions [16s, 16s+16) of the result are meaningful; the selector
    matmul extracts them (DVE cost is free-size driven, partitions are
    parallel lanes, and 16-partition slices at odd 16-offsets violate the
    32-alignment rule)."""
    sl = vt[:, 0, c0:c0 + GW * 128 * Kgs]
    rap = bass.AP(
        sl.tensor, sl.offset,
        [list(sl.ap[0]), [1, GW * 128], [GW * 128, Kgs]],
    )
    nc.vector.tensor_reduce(
        out=Ps[:], in_=rap,
        axis=mybir.AxisListType.X, op=mybir.AluOpType.add,
    )


def _build_BC(plan, layer):
    nwin, ngrp, nloc, nwt = (plan["nwin"], plan["ngrp"], plan["nloc"],
                             plan["nwt"])
    K, offs, Cg, grpbase, split, T_dev = (plan["K"], plan["offs"], plan["Cg"],
                                          plan["grpbase"], plan["split"],
                                          plan["T_dev"])
    WB = GW * 128
    cg_max = max(int(c) for c in Cg)

    nc = bacc.Bacc("TRN2", target_bir_lowering=False, debug=False,
                   num_devices=NDEV, num_swdge_queues=NQUEUES,
                   dynamic_dma_scratch_size=DMASCRATCH)
    tab_d = nc.dram_tensor("tab", [128, (nwt + 1) * 128], F16,
                           kind="ExternalInput").ap()
    idx_d = nc.dram_tensor("idx", [128, T_dev // 16], I16,
                           kind="ExternalInput").ap()
    dinvw_d = nc.dram_tensor("dinvw", [64, nloc], F16,
                             kind="ExternalInput").ap()
    if layer == 1:
        b_d = nc.dram_tensor("b1", [HID, 1], F32, kind="ExternalInput").ap()
        ts_d = nc.dram_tensor("tself", [HID, nloc], F16,
                              kind="ExternalInput").ap()
        sel_d = nc.dram_tensor("sel", [128, NCLS, HID], F32,
                               kind="ExternalInput").ap()
        id_d = nc.dram_tensor("ident", [HID, HID], F16,
                              kind="ExternalInput").ap()
        o_d = nc.dram_tensor("t2", [nloc, HID], F16,
                             kind="ExternalOutput").ap()
        ov = o_d.rearrange("(w p) f -> p w f", p=128)
    else:
        b_d = nc.dram_tensor("b2", [DOUT, 1], F32, kind="ExternalInput").ap()
        ts_d = nc.dram_tensor("wself", [DOUT, nloc], F16,
                              kind="ExternalInput").ap()
        sel_d = nc.dram_tensor("selw2", [128, NCLS, DOUT], F32,
                               kind="ExternalInput").ap()
        o_d = nc.dram_tensor("o2", [DOUT, nloc], F32,
                             kind="ExternalOutput").ap()
        ov = o_d

    with tile.TileContext(nc) as tc:
        with (
            tc.tile_pool(name="cst", bufs=1) as cst,
            tc.tile_pool(name="ip", bufs=3) as ip,
            tc.tile_pool(name="vp", bufs=2) as vp,
            tc.tile_pool(name="pp", bufs=3) as pp,
            tc.tile_pool(name="sm", bufs=3) as sm,
            tc.tile_pool(name="pY", bufs=2, space="PSUM") as pY,
            tc.tile_pool(name="pT", bufs=2, space="PSUM") as pT,
            tc.tile_pool(name="stg", bufs=2) as stg,
        ):
            nc.gpsimd.load_library(library_config.mlp)
            tabt = cst.tile([128, (nwt + 1) * 128], F16)
            nc.sync.dma_start(out=tabt[:], in_=tab_d[:])
            dvw = cst.tile([64, nloc], F16)
            nc.sync.dma_start(out=dvw[:], in_=dinvw_d[:])
            nb_ = HID if layer == 1 else DOUT
            bt = cst.tile([nb_, 1], F32)
            nc.sync.dma_start(out=bt[:], in_=b_d[:])
            tsl = cst.tile([nb_, nloc], F16)
            nc.sync.dma_start(out=tsl[:], in_=ts_d[:])
            selt = cst.tile([128, NCLS, nb_], F32)
            nc.sync.dma_start(out=selt[:], in_=sel_d[:])
            if layer == 1:
                idt = cst.tile([HID, HID], F16)
                nc.sync.dma_start(out=idt[:], in_=id_d[:])

            it_max = cg_max // 16
            qctr = 0
            for g in range(ngrp):
                cg = int(Cg[g])
                t0 = int(grpbase[g])
                it = ip.tile([128, it_max], I16, tag="idx")
                nc.sync.dma_start(out=it[:, :cg // 16],
                                  in_=idx_d[:, t0 // 16:(t0 + cg) // 16])
                va = vp.tile([128, 1, cg_max], F16, tag="va")
                # Rotate SWDGE queues: queue q runs on Q7 cores 2q/2q+1 with
                # its own descriptor ring, so gather N+1's descgen overlaps
                # gather N's SDMA drain (measured 8.1 -> 2.3 ns/idx at nq=4).
                nch = (cg + GCHUNK - 1) // GCHUNK
                bnds = [(cg * i // nch) // 128 * 128 for i in range(nch + 1)]
                bnds[-1] = cg
                for c0, c1 in zip(bnds[:-1], bnds[1:]):
                    _sbuf_gather(nc, va[:, :, c0:c1], tabt,
                                 it[:, c0 // 16:c1 // 16], c1 - c0,
                                 qctr % NQUEUES)
                    qctr += 1
                Y = pY.tile([nb_, WB], F32)
                for s in range(NCLS):
                    Kgs = int(K[g, s])
                    Ps = pp.tile([128, WB], F32, tag=f"P{s}")
                    _reduce_cls(nc, Ps, va, int(offs[g, s]), Kgs)
                    nc.tensor.matmul(out=Y[:], lhsT=selt[:, s, :], rhs=Ps[:],
                                     start=(s == 0), stop=(s == NCLS - 1))
                dsl = dvw[0:nb_, g * WB:(g + 1) * WB]
                ya = sm.tile([nb_, WB], F32, tag="ya")
                nc.vector.scalar_tensor_tensor(
                    out=ya[:], in0=Y[:], scalar=1.0,
                    in1=tsl[:, g * WB:(g + 1) * WB],
                    op0=mybir.AluOpType.mult, op1=mybir.AluOpType.add,
                )
                if layer == 1:
                    yd = sm.tile([HID, WB], F32, tag="yd")
                    nc.vector.scalar_tensor_tensor(
                        out=yd[:], in0=ya[:], scalar=1.0, in1=dsl,
                        op0=mybir.AluOpType.mult, op1=mybir.AluOpType.mult,
                    )
                    r = sm.tile([HID, WB], F32, tag="r")
                    nc.scalar.activation(
                        out=r[:], in_=yd[:],
                        func=mybir.ActivationFunctionType.Relu,
                        bias=bt[:, 0:1],
                    )
                    t2c = sm.tile([HID, WB], F16, tag="t2c")
                    nc.vector.scalar_tensor_tensor(
                        out=t2c[:], in0=r[:], scalar=1.0, in1=dsl,
                        op0=mybir.AluOpType.mult, op1=mybir.AluOpType.mult,
                    )
                    stage = stg.tile([128, GW, HID], F16)
                    for j in range(GW):
                        tp = pT.tile([128, HID], F16)
                        nc.tensor.transpose(
                            out=tp[:], in_=t2c[:, j * 128:(j + 1) * 128],
                            identity=idt[:])
                        nc.vector.tensor_copy(out=stage[:, j, :], in_=tp[:])
                    nc.sync.dma_start(out=ov[:, g * GW:(g + 1) * GW, :],
                                      in_=stage[:])
                else:
                    od = sm.tile([DOUT, WB], F32, tag="od")
                    nc.vector.scalar_tensor_tensor(
                        out=od[:], in0=ya[:], scalar=1.0, in1=dsl,
                        op0=mybir.AluOpType.mult, op1=mybir.AluOpType.mult,
                    )
                    stage = stg.tile([DOUT, WB], F32)
                    nc.vector.tensor_scalar_add(
                        out=stage[:], in0=od[:], scalar1=bt[:, 0:1],
                    )
                    nc.sync.dma_start(out=ov[:, g * WB:(g + 1) * WB],
                                      in_=stage[:])
    nc.compile()
    return nc


# ----------------------------------------------------------------------------
# driver
# ----------------------------------------------------------------------------

_PROG_CACHE = {}


def _run(nc, in_maps):
    trace = os.environ.get("GCN_TRACE", "0") == "1"
    res = bass_utils.run_bass_kernel_spmd(
        nc, in_maps, core_ids=list(range(NDEV)), trace=trace)
    if res.exec_time_ns is not None:
        LAST_EXEC_NS.append(int(res.exec_time_ns))
    return res.results


def _build_table(plan, t_node):
    nwt = plan["nwt"]
    tab = np.zeros((128, nwt + 1, NCLS, HID), np.float16)
    q, s, u = plan["slot_q"], plan["slot_s"], plan["slot_u"]
    tab[q % 128, q // 128, s] = t_node[u]
    return np.ascontiguousarray(tab.reshape(128, (nwt + 1) * 128))


def kernel(x, edge_index, W1, b1, W2, b2):
    LAST_EXEC_NS.clear()
    x = np.asarray(x, np.float32)
    W1 = np.asarray(W1, np.float32)
    b1 = np.asarray(b1, np.float32)
    W2 = np.asarray(W2, np.float32)
    b2 = np.asarray(b2, np.float32)

    ei = np.asarray(edge_index)
    plan = _plan(ei, seed=12345)
    key = ("v5", plan["nwin"], plan["T_dev"], int(plan["K"].sum()))
    if key not in _PROG_CACHE:
        _PROG_CACHE.clear()
        _PROG_CACHE[key] = (_build_A(plan), _build_BC(plan, 1),
                            _build_BC(plan, 2))
    ncA, ncB, ncC = _PROG_CACHE[key]

    ridx, nor = plan["ridx"], plan["node_of_rank"]
    npad, nloc = plan["npad"], plan["nloc"]

    xfull = np.zeros((npad, DIN), np.float32)
    xfull[plan["rank_n"]] = x
    w1r = np.ascontiguousarray(
        W1.reshape(2, 128, HID).transpose(1, 0, 2))
    inA = [{"xT": np.ascontiguousarray(xfull[ridx[d]].T),
            "dinva": plan["dinva"][d], "w1": w1r} for d in range(NDEV)]
    resA = _run(ncA, inA)
    t1n = np.zeros((N, HID), np.float16)
    for d in range(NDEV):
        m = nor[ridx[d]] >= 0
        t1n[nor[ridx[d]][m]] = resA[d]["t1"][m]

    sel = np.zeros((128, NCLS, HID), np.float32)
    for s in range(NCLS):
        sel[16 * s + np.arange(HID), s, np.arange(HID)] = 1.0
    def _self_arr(tn):
        out = []
        for d in range(NDEV):
            a = np.zeros((nloc, tn.shape[1]), np.float16)
            m = nor[ridx[d]] >= 0
            a[m] = tn[nor[ridx[d]][m]]
            out.append(np.ascontiguousarray(a.T))
        return out

    ts1 = _self_arr(t1n)
    inB = [{"tab": _build_table(plan, t1n), "idx": plan["idxw"][d],
            "dinvw": plan["dinvw"][d], "b1": b1[:, None].astype(np.float32),
            "tself": ts1[d], "sel": sel,
            "ident": np.eye(HID, dtype=np.float16)}
           for d in range(NDEV)]
    resB = _run(ncB, inB)
    t2n = np.zeros((N, HID), np.float16)
    for d in range(NDEV):
        m = nor[ridx[d]] >= 0
        t2n[nor[ridx[d]][m]] = resB[d]["t2"][m]

    selw2 = np.zeros((128, NCLS, DOUT), np.float32)
    for s in range(NCLS):
        selw2[16 * s + np.arange(HID), s, :] = W2
    ws = _self_arr((t2n.astype(np.float32) @ W2).astype(np.float16))
    inC = [{"tab": _build_table(plan, t2n), "idx": plan["idxw"][d],
            "dinvw": plan["dinvw"][d], "b2": b2[:, None].astype(np.float32),
            "wself": ws[d], "selw2": selw2} for d in range(NDEV)]
    resC = _run(ncC, inC)
    out = np.zeros((N, DOUT), np.float32)
    for d in range(NDEV):
        m = nor[ridx[d]] >= 0
        out[nor[ridx[d]][m]] = resC[d]["o2"].T[m]
    return out



# revision 10
# speedup vs baseline: 1.1991x; 1.0773x over previous
"""Trainium2 Bass kernel for 2-layer GCN (GCNConv -> ReLU -> GCNConv).

v2 strategy — SBUF-resident fp16 tables + transpose-mode SBUF-source gathers
(the baseline's HBM dma_gather was HBM-latency bound at ~63ns/edge):

- Both layers reduce to: gather 16-wide rows t[src], segment-sum by dst
  (linear layers commute with the normalized aggregation).
- The 16-fp16 (32B) node records live in SBUF as [128 part, W windows] of
  256B chunks; chunk (tok, w) holds 8 records at positions s=0..7.
- A token (edge) gathers its source's 256B chunk via dma_gather(transpose=
  True, SBUF source): the chunk becomes a 128-partition fp16 column; the
  wanted record sits at partition slice [16s, 16s+16) where s = the record's
  chunk position ("class").  Chunk-mates land on other slices — never read.
- Host assigns each node TWO candidate classes and each edge picks one
  (power-of-two-choices), balancing per-(destination-group, class) slot
  counts K.  Grid columns per group g: [class s][slot k<K[g,s]][win j][dst p]
  so one strided DVE tensor_reduce per (group, class) segment-sums slot
  layers into slice s of a [128, 256] tile P.  P's 8 slices collapse via a
  PE matmul with a 0/1 selector (layer 2 fuses W2 into the selector).
- 3 SPMD launches: A (t1 = dinv*x@W1), B (layer-1 aggregate -> t2),
  C (layer-2 aggregate -> @W2+b2).  Host re-shards tables between launches.
"""
import os
import sys

sys.path.insert(0, "/opt/trn_rl_repo")

import numpy as np

import concourse.bass as bass
import concourse.mybir as mybir
import concourse.tile as tile
from concourse import bacc, bass_utils, library_config

N = 100000
E = 1600000
DIN, HID, DOUT = 256, 16, 64
NDEV = 8
NCLS = 8
GW = 1                      # windows per K-uniform group
F32 = mybir.dt.float32
F16 = mybir.dt.float16
I16 = mybir.dt.int16
NQUEUES = int(os.environ.get("GCN_NQUEUES", "1"))
GCHUNK = int(os.environ.get("GCN_GCHUNK", "2048"))
DMASCRATCH = int(os.environ.get("GCN_DMASCRATCH", "32768"))
N3 = int(os.environ.get("GCN_N3", "62000"))
MAXSLOTS = 32640              # 255 windows/class keeps idx_pad in int16

LAST_EXEC_NS = []


# ----------------------------------------------------------------------------
# host-side graph planning
# ----------------------------------------------------------------------------

def _ragged_arange(lens):
    ends = np.cumsum(lens)
    total = int(ends[-1]) if len(lens) else 0
    out = np.arange(total, dtype=np.int64)
    out -= np.repeat(ends - lens, lens)
    return out


def _plan(edge_index, seed=12345):
    rng = np.random.default_rng(seed)
    src = np.asarray(edge_index[0], dtype=np.int64)
    dst = np.asarray(edge_index[1], dtype=np.int64)
    # self-loops are NOT tokens: the dst's own record is added in postproc
    all_src = src
    all_dst = dst
    T = len(all_src)
    indeg = np.bincount(dst, minlength=N).astype(np.int64) + 1  # GCN degree
    dinv_n = (1.0 / np.sqrt(indeg.astype(np.float64))).astype(np.float32)

    # rank deal: degree-sorted; i-th -> device i%8, window (i//8)//128
    order = np.argsort(-indeg, kind="stable")
    di = np.empty(N, np.int64)
    di[order] = np.arange(N)
    dev_n = di % NDEV
    w_n = (di // NDEV) // 128
    p_n = (di // NDEV) % 128
    nwin = int(w_n.max()) + 1
    ngrp = (nwin + GW - 1) // GW
    nwin_pad = ngrp * GW
    rank_n = (w_n * NDEV + dev_n) * 128 + p_n
    nloc = nwin_pad * 128
    npad = nloc * NDEV

    grp_n = w_n // GW

    # --- class assignment: mixed-R candidates + balanced greedy + repair ---
    ko = np.argsort(all_dst, kind="stable")
    t_dst = all_dst[ko]
    t_src = all_src[ko]
    tg = grp_n[t_dst]

    def _make_cand(n3):
        s1 = rng.integers(0, NCLS, N)
        s2 = (s1 + 1 + rng.integers(0, NCLS - 1, N)) % NCLS
        cand = np.stack([s1, s2], 1)
        if n3 > 0:
            odeg = np.bincount(all_src, minlength=N)
            top = np.argsort(-odeg, kind="stable")[:n3]
            s3 = rng.integers(0, NCLS, n3)
            bad = (s3 == s1[top]) | (s3 == s2[top])
            while bad.any():
                s3[bad] = rng.integers(0, NCLS, bad.sum())
                bad = (s3 == s1[top]) | (s3 == s2[top])
            c3 = np.full(N, -1, np.int64)
            c3[top] = s3
            cand = np.concatenate([cand, c3[:, None]], 1)
        return cand

    starts = np.searchsorted(t_dst, np.arange(N + 1))
    pos = np.arange(T) - np.repeat(starts[:-1], np.diff(starts))

    def _greedy(cand):
        cnt = np.zeros((N, NCLS), np.int16)
        cls_tok = np.zeros(T, np.int8)
        for k in range(int(pos.max()) + 1):
            m = np.flatnonzero(pos == k)
            if len(m) == 0:
                break
            u, v = t_src[m], t_dst[m]
            cc = cand[u]
            counts = np.where(cc >= 0, cnt[v[:, None], np.maximum(cc, 0)], 127)
            best = np.argmin(counts, axis=1)
            c = cc[np.arange(len(m)), best]
            cls_tok[m] = c
            cnt[v, c] += 1
        return cls_tok, cnt

    def _repair(cls_tok, cnt, cand, iters=400):
        R = cand.shape[1]
        cls_tok = cls_tok.astype(np.int64)
        tc = cand[t_src]
        for it in range(iters):
            K = np.zeros((ngrp, NCLS), np.int64)
            np.maximum.at(K, (tg, cls_tok), cnt[t_dst, cls_tok])
            K = np.maximum(K, 1)
            cur = cls_tok
            crit = cnt[t_dst, cur] == K[tg, cur]
            alt_ok = np.full(T, -1, np.int64)
            for r in rng.permutation(R):
                a = tc[:, r]
                a0 = np.maximum(a, 0)
                ok = (crit & (a >= 0) & (a != cur) & (alt_ok < 0)
                      & (cnt[t_dst, a0] + 1 < K[tg, a0]))
                alt_ok[ok] = a[ok]
            mv = np.flatnonzero(alt_ok >= 0)
            if len(mv) == 0:
                # 2-chain: evict a non-critical blocker from (dst, a) where a
                # sits one below the group max, freeing room for a critical
                # token to move there next iteration.
                relief = np.full(T, -1, np.int64)
                for r in rng.permutation(R):
                    a = tc[:, r]
                    a0 = np.maximum(a, 0)
                    ok = (crit & (a >= 0) & (a != cur) & (relief < 0)
                          & (cnt[t_dst, a0] + 1 == K[tg, a0])
                          & (cnt[t_dst, a0] > 0))
                    relief[ok] = a[ok]
                want = np.flatnonzero(relief >= 0)
                if len(want) == 0:
                    break
                wantset = np.zeros(N * NCLS, bool)
                wantset[t_dst[want] * NCLS + relief[want]] = True
                isblk = wantset[t_dst * NCLS + cur] & ~crit
                blk_alt = np.full(T, -1, np.int64)
                for r in rng.permutation(R):
                    a = tc[:, r]
                    a0 = np.maximum(a, 0)
                    ok = (isblk & (a >= 0) & (a != cur) & (blk_alt < 0)
                          & (cnt[t_dst, a0] + 1 < K[tg, a0]))
                    blk_alt[ok] = a[ok]
                mv = np.flatnonzero(blk_alt >= 0)
                if len(mv) == 0:
                    break
                alt_ok = blk_alt
            ks = np.argsort(t_dst[mv], kind="stable")
            kk = t_dst[mv][ks]
            first = np.concatenate([[True], kk[1:] != kk[:-1]])
            mv = mv[ks[first]]
            cv, av, vv = cur[mv].copy(), alt_ok[mv], t_dst[mv]
            cls_tok[mv] = av
            np.subtract.at(cnt, (vv, cv), 1)
            np.add.at(cnt, (vv, av), 1)
        K = np.zeros((ngrp, NCLS), np.int64)
        np.maximum.at(K, (tg, cls_tok), cnt[t_dst, cls_tok])
        return cls_tok.astype(np.int8), cnt, np.maximum(K, 1)

    n3 = N3
    while True:
        cand = _make_cand(n3)
        cls_tok, cnt = _greedy(cand)
        cls_tok, cnt, K = _repair(cls_tok, cnt, cand)
        used_chk = np.zeros((N, NCLS), bool)
        used_chk[t_src, cls_tok.astype(np.int64)] = True
        if used_chk.sum(0).max() <= MAXSLOTS or n3 == 0:
            break
        n3 = max(0, n3 - 15000)

    # --- table slot allocation per class ---
    used = np.zeros((N, NCLS), bool)
    used[t_src, cls_tok.astype(np.int64)] = True
    gidx = np.zeros((N, NCLS), np.int32)
    slot_u, slot_s, slot_q = [], [], []
    nwt = 0
    for s in range(NCLS):
        us = np.flatnonzero(used[:, s])
        q = np.arange(len(us))
        gidx[us, s] = (q // 128) * 128 + (q % 128)
        slot_u.append(us)
        slot_s.append(np.full(len(us), s))
        slot_q.append(q)
        nwt = max(nwt, (len(us) + 127) // 128)
    slot_u = np.concatenate(slot_u)
    slot_s = np.concatenate(slot_s)
    slot_q = np.concatenate(slot_q)
    idx_pad = nwt * 128                    # zero window
    assert idx_pad + 127 < 32768

    # --- grid column layout (global K; identical on all devices) ---
    offs = np.concatenate([np.zeros((ngrp, 1), np.int64),
                           np.cumsum(K, axis=1)], axis=1) * (GW * 128)
    Cg = offs[:, -1]
    grpbase = np.concatenate([[0], np.cumsum(Cg)])
    T_dev = int(grpbase[-1])
    assert T_dev % 16 == 0

    # gather split per group: class boundary nearest the middle
    split = []
    for g in range(ngrp):
        sh = int(np.argmin(np.abs(offs[g, 1:-1] - Cg[g] / 2))) + 1
        split.append((sh, int(offs[g, sh])))

    # --- token -> column, idx arrays per device ---
    occ = np.empty(T, np.int64)
    key2 = t_dst * NCLS + cls_tok.astype(np.int64)
    k2o = np.argsort(key2, kind="stable")
    kk2 = key2[k2o]
    bnd = np.concatenate([[True], kk2[1:] != kk2[:-1]])
    gstarts = np.flatnonzero(bnd)
    glens = np.diff(np.concatenate([gstarts, [T]]))
    occ[k2o] = _ragged_arange(glens)

    v = t_dst
    g = grp_n[v]
    col = (grpbase[g] + offs[g, cls_tok.astype(np.int64)]
           + occ * (GW * 128) + (w_n[v] % GW) * 128 + p_n[v])
    tdev = dev_n[v]
    srcval = gidx[t_src, cls_tok.astype(np.int64)].astype(np.int16)
    idxw = np.empty((NDEV, 128, T_dev // 16), np.int16)
    for d in range(NDEV):
        m = tdev == d
        a = np.full(T_dev, idx_pad, np.int16)
        a[col[m]] = srcval[m]
        idxw[d] = np.tile(a.reshape(T_dev // 16, 16).T, (8, 1))

    # --- per-device aux arrays ---
    ridx = np.empty((NDEV, nloc), np.int64)
    for d in range(NDEV):
        gg = ((np.arange(nwin_pad) * NDEV + d)[:, None] * 128 + np.arange(128))
        ridx[d] = gg.reshape(-1)
    node_of_rank = np.full(npad, -1, np.int64)
    node_of_rank[rank_n] = np.arange(N)
    dinv_r = np.zeros(npad, np.float32)
    dinv_r[rank_n] = dinv_n

    dinva = np.empty((NDEV, 128, nwin_pad), np.float32)
    dinvw = np.empty((NDEV, 64, nloc), np.float16)
    for d in range(NDEV):
        dr = dinv_r[ridx[d]]
        dinva[d] = dr.reshape(nwin_pad, 128).T
        dinvw[d] = np.tile(dr[None, :], (64, 1)).astype(np.float16)

    return dict(
        nwin=nwin_pad, ngrp=ngrp, nloc=nloc, npad=npad, nwt=nwt,
        idx_pad=idx_pad, K=K, offs=offs, Cg=Cg, grpbase=grpbase,
        T_dev=T_dev, split=split, idxw=idxw, ridx=ridx,
        node_of_rank=node_of_rank, rank_n=rank_n, dinva=dinva, dinvw=dinvw,
        slot_u=slot_u, slot_s=slot_s, slot_q=slot_q,
    )


# ----------------------------------------------------------------------------
# device programs
# ----------------------------------------------------------------------------

def _build_A(plan):
    nwin, nloc = plan["nwin"], plan["nloc"]
    nc = bacc.Bacc("TRN2", target_bir_lowering=False, debug=False,
                   num_devices=NDEV)
    xT_d = nc.dram_tensor("xT", [DIN, nloc], F32, kind="ExternalInput").ap()
    dinva_d = nc.dram_tensor("dinva", [128, nwin], F32,
                             kind="ExternalInput").ap()
    w1_d = nc.dram_tensor("w1", [128, 2, HID], F32, kind="ExternalInput").ap()
    t1_d = nc.dram_tensor("t1", [nloc, HID], F16, kind="ExternalOutput").ap()

    with tile.TileContext(nc) as tc:
        with (
            tc.tile_pool(name="cst", bufs=1) as cst,
            tc.tile_pool(name="xp", bufs=3) as xp,
            tc.tile_pool(name="ps", bufs=2, space="PSUM") as psp,
            tc.tile_pool(name="stg", bufs=2) as stg,
        ):
            w1t = cst.tile([128, 2, HID], F32)
            nc.sync.dma_start(out=w1t[:], in_=w1_d[:])
            dat = cst.tile([128, nwin], F32)
            nc.sync.dma_start(out=dat[:], in_=dinva_d[:])
            ov = t1_d.rearrange("(w p) f -> p w f", p=128)
            for i0 in range(0, nwin, 8):
                nb = min(8, nwin - i0)
                xts = []
                for k in range(2):
                    xt = xp.tile([128, 8 * 128], F32, tag=f"xt{k}")
                    nc.sync.dma_start(
                        out=xt[:, :nb * 128],
                        in_=xT_d[k * 128:(k + 1) * 128,
                                 i0 * 128:(i0 + nb) * 128],
                    )
                    xts.append(xt)
                stage = stg.tile([128, 8, HID], F16)
                for ib in range(nb):
                    ps = psp.tile([128, HID], F32)
                    for k in range(2):
                        nc.tensor.matmul(
                            out=ps[:],
                            lhsT=xts[k][:, ib * 128:(ib + 1) * 128],
                            rhs=w1t[:, k, :],
                            start=(k == 0), stop=(k == 1),
                        )
                    nc.vector.tensor_scalar_mul(
                        out=stage[:, ib, :], in0=ps[:],
                        scalar1=dat[:, i0 + ib:i0 + ib + 1],
                    )
                nc.sync.dma_start(out=ov[:, i0:i0 + nb, :],
                                  in_=stage[:, :nb, :])
    nc.compile()
    return nc


def _sbuf_gather(nc, out_sl, tabt, idx_sl, ncols, q):
    nc.gpsimd.dma_gather(
        out_sl, tabt[:], idx_sl,
        num_idxs=ncols, num_idxs_reg=ncols, elem_size=128,
        transpose=True, single_packet=False, queue_num=q,
        sbuf_tokens_per_rank=128, sbuf_free_dim_per_rank=256,
        sbuf_free_dim_pad_per_rank=0, sbuf_byte_offset=0,
    )


def _reduce_cls(nc, Ps, vt, c0, Kgs):
    """Full-128-partition strided segment-sum of one class's token columns.
    Only partitions [16s, 16s+16) of the result are meaningful; the selector
    matmul extracts them (DVE cost is free-size driven, partitions are
    parallel lanes, and 16-partition slices at odd 16-offsets violate the
    32-alignment rule)."""
    sl = vt[:, 0, c0:c0 + GW * 128 * Kgs]
    rap = bass.AP(
        sl.tensor, sl.offset,
        [list(sl.ap[0]), [1, GW * 128], [GW * 128, Kgs]],
    )
    nc.vector.tensor_reduce(
        out=Ps[:], in_=rap,
        axis=mybir.AxisListType.X, op=mybir.AluOpType.add,
    )


def _build_BC(plan, layer):
    nwin, ngrp, nloc, nwt = (plan["nwin"], plan["ngrp"], plan["nloc"],
                             plan["nwt"])
    K, offs, Cg, grpbase, split, T_dev = (plan["K"], plan["offs"], plan["Cg"],
                                          plan["grpbase"], plan["split"],
                                          plan["T_dev"])
    WB = GW * 128
    cg_max = max(int(c) for c in Cg)

    nc = bacc.Bacc("TRN2", target_bir_lowering=False, debug=False,
                   num_devices=NDEV, num_swdge_queues=NQUEUES,
                   dynamic_dma_scratch_size=DMASCRATCH)
    tab_d = nc.dram_tensor("tab", [128, (nwt + 1) * 128], F16,
                           kind="ExternalInput").ap()
    idx_d = nc.dram_tensor("idx", [128, T_dev // 16], I16,
                           kind="ExternalInput").ap()
    dinvw_d = nc.dram_tensor("dinvw", [64, nloc], F16,
                             kind="ExternalInput").ap()
    if layer == 1:
        b_d = nc.dram_tensor("b1", [HID, 1], F32, kind="ExternalInput").ap()
        ts_d = nc.dram_tensor("tself", [HID, nloc], F16,
                              kind="ExternalInput").ap()
        sel_d = nc.dram_tensor("sel", [128, NCLS, HID], F32,
                               kind="ExternalInput").ap()
        id_d = nc.dram_tensor("ident", [HID, HID], F16,
                              kind="ExternalInput").ap()
        o_d = nc.dram_tensor("t2", [nloc, HID], F16,
                             kind="ExternalOutput").ap()
        ov = o_d.rearrange("(w p) f -> p w f", p=128)
    else:
        b_d = nc.dram_tensor("b2", [DOUT, 1], F32, kind="ExternalInput").ap()
        ts_d = nc.dram_tensor("wself", [DOUT, nloc], F16,
                              kind="ExternalInput").ap()
        sel_d = nc.dram_tensor("selw2", [128, NCLS, DOUT], F32,
                               kind="ExternalInput").ap()
        o_d = nc.dram_tensor("o2", [DOUT, nloc], F32,
                             kind="ExternalOutput").ap()
        ov = o_d

    with tile.TileContext(nc) as tc:
        with (
            tc.tile_pool(name="cst", bufs=1) as cst,
            tc.tile_pool(name="ip", bufs=3) as ip,
            tc.tile_pool(name="vp", bufs=2) as vp,
            tc.tile_pool(name="pp", bufs=3) as pp,
            tc.tile_pool(name="sm", bufs=3) as sm,
            tc.tile_pool(name="pY", bufs=2, space="PSUM") as pY,
            tc.tile_pool(name="pT", bufs=2, space="PSUM") as pT,
            tc.tile_pool(name="stg", bufs=2) as stg,
        ):
            nc.gpsimd.load_library(library_config.mlp)
            tabt = cst.tile([128, (nwt + 1) * 128], F16)
            nc.sync.dma_start(out=tabt[:], in_=tab_d[:])
            dvw = cst.tile([64, nloc], F16)
            nc.sync.dma_start(out=dvw[:], in_=dinvw_d[:])
            nb_ = HID if layer == 1 else DOUT
            bt = cst.tile([nb_, 1], F32)
            nc.sync.dma_start(out=bt[:], in_=b_d[:])
            tsl = cst.tile([nb_, nloc], F16)
            nc.sync.dma_start(out=tsl[:], in_=ts_d[:])
            selt = cst.tile([128, NCLS, nb_], F32)
            nc.sync.dma_start(out=selt[:], in_=sel_d[:])
            if layer == 1:
                idt = cst.tile([HID, HID], F16)
                nc.sync.dma_start(out=idt[:], in_=id_d[:])

            it_max = cg_max // 16
            qctr = 0
            for g in range(ngrp):
                cg = int(Cg[g])
                t0 = int(grpbase[g])
                it = ip.tile([128, it_max], I16, tag="idx")
                nc.sync.dma_start(out=it[:, :cg // 16],
                                  in_=idx_d[:, t0 // 16:(t0 + cg) // 16])
                va = vp.tile([128, 1, cg_max], F16, tag="va")
                # Rotate SWDGE queues: queue q runs on Q7 cores 2q/2q+1 with
                # its own descriptor ring, so gather N+1's descgen overlaps
                # gather N's SDMA drain (measured 8.1 -> 2.3 ns/idx at nq=4).
                nch = (cg + GCHUNK - 1) // GCHUNK
                bnds = [(cg * i // nch) // 128 * 128 for i in range(nch + 1)]
                bnds[-1] = cg
                for c0, c1 in zip(bnds[:-1], bnds[1:]):
                    _sbuf_gather(nc, va[:, :, c0:c1], tabt,
                                 it[:, c0 // 16:c1 // 16], c1 - c0,
                                 qctr % NQUEUES)
                    qctr += 1
                Y = pY.tile([nb_, WB], F32)
                for s in range(NCLS):
                    Kgs = int(K[g, s])
                    Ps = pp.tile([128, WB], F32, tag=f"P{s}")
                    _reduce_cls(nc, Ps, va, int(offs[g, s]), Kgs)
                    nc.tensor.matmul(out=Y[:], lhsT=selt[:, s, :], rhs=Ps[:],
                                     start=(s == 0), stop=(s == NCLS - 1))
                dsl = dvw[0:nb_, g * WB:(g + 1) * WB]
                ya = sm.tile([nb_, WB], F32, tag="ya")
                nc.vector.scalar_tensor_tensor(
                    out=ya[:], in0=Y[:], scalar=1.0,
                    in1=tsl[:, g * WB:(g + 1) * WB],
                    op0=mybir.AluOpType.mult, op1=mybir.AluOpType.add,
                )
                if layer == 1:
                    yd = sm.tile([HID, WB], F32, tag="yd")
                    nc.vector.scalar_tensor_tensor(
                        out=yd[:], in0=ya[:], scalar=1.0, in1=dsl,
                        op0=mybir.AluOpType.mult, op1=mybir.AluOpType.mult,
                    )
                    r = sm.tile([HID, WB], F32, tag="r")
                    nc.scalar.activation(
                        out=r[:], in_=yd[:],
                        func=mybir.ActivationFunctionType.Relu,
                        bias=bt[:, 0:1],
                    )
                    t2c = sm.tile([HID, WB], F16, tag="t2c")
                    nc.vector.scalar_tensor_tensor(
                        out=t2c[:], in0=r[:], scalar=1.0, in1=dsl,
                        op0=mybir.AluOpType.mult, op1=mybir.AluOpType.mult,
                    )
                    stage = stg.tile([128, GW, HID], F16)
                    for j in range(GW):
                        tp = pT.tile([128, HID], F16)
                        nc.tensor.transpose(
                            out=tp[:], in_=t2c[:, j * 128:(j + 1) * 128],
                            identity=idt[:])
                        nc.vector.tensor_copy(out=stage[:, j, :], in_=tp[:])
                    nc.sync.dma_start(out=ov[:, g * GW:(g + 1) * GW, :],
                                      in_=stage[:])
                else:
                    od = sm.tile([DOUT, WB], F32, tag="od")
                    nc.vector.scalar_tensor_tensor(
                        out=od[:], in0=ya[:], scalar=1.0, in1=dsl,
                        op0=mybir.AluOpType.mult, op1=mybir.AluOpType.mult,
                    )
                    stage = stg.tile([DOUT, WB], F32)
                    nc.vector.tensor_scalar_add(
                        out=stage[:], in0=od[:], scalar1=bt[:, 0:1],
                    )
                    nc.sync.dma_start(out=ov[:, g * WB:(g + 1) * WB],
                                      in_=stage[:])
    nc.compile()
    return nc


# ----------------------------------------------------------------------------
# driver
# ----------------------------------------------------------------------------

_PROG_CACHE = {}


def _run(nc, in_maps):
    trace = os.environ.get("GCN_TRACE", "0") == "1"
    res = bass_utils.run_bass_kernel_spmd(
        nc, in_maps, core_ids=list(range(NDEV)), trace=trace)
    if res.exec_time_ns is not None:
        LAST_EXEC_NS.append(int(res.exec_time_ns))
    return res.results


def _build_table(plan, t_node):
    nwt = plan["nwt"]
    tab = np.zeros((128, nwt + 1, NCLS, HID), np.float16)
    q, s, u = plan["slot_q"], plan["slot_s"], plan["slot_u"]
    tab[q % 128, q // 128, s] = t_node[u]
    return np.ascontiguousarray(tab.reshape(128, (nwt + 1) * 128))


def kernel(x, edge_index, W1, b1, W2, b2):
    LAST_EXEC_NS.clear()
    x = np.asarray(x, np.float32)
    W1 = np.asarray(W1, np.float32)
    b1 = np.asarray(b1, np.float32)
    W2 = np.asarray(W2, np.float32)
    b2 = np.asarray(b2, np.float32)

    ei = np.asarray(edge_index)
    plan = _plan(ei, seed=7)
    key = ("v6", plan["nwin"], plan["T_dev"], int(plan["K"].sum()))
    if key not in _PROG_CACHE:
        _PROG_CACHE.clear()
        _PROG_CACHE[key] = (_build_A(plan), _build_BC(plan, 1),
                            _build_BC(plan, 2))
    ncA, ncB, ncC = _PROG_CACHE[key]

    ridx, nor = plan["ridx"], plan["node_of_rank"]
    npad, nloc = plan["npad"], plan["nloc"]

    xfull = np.zeros((npad, DIN), np.float32)
    xfull[plan["rank_n"]] = x
    w1r = np.ascontiguousarray(
        W1.reshape(2, 128, HID).transpose(1, 0, 2))
    inA = [{"xT": np.ascontiguousarray(xfull[ridx[d]].T),
            "dinva": plan["dinva"][d], "w1": w1r} for d in range(NDEV)]
    resA = _run(ncA, inA)
    t1n = np.zeros((N, HID), np.float16)
    for d in range(NDEV):
        m = nor[ridx[d]] >= 0
        t1n[nor[ridx[d]][m]] = resA[d]["t1"][m]

    sel = np.zeros((128, NCLS, HID), np.float32)
    for s in range(NCLS):
        sel[16 * s + np.arange(HID), s, np.arange(HID)] = 1.0
    def _self_arr(tn):
        out = []
        for d in range(NDEV):
            a = np.zeros((nloc, tn.shape[1]), np.float16)
            m = nor[ridx[d]] >= 0
            a[m] = tn[nor[ridx[d]][m]]
            out.append(np.ascontiguousarray(a.T))
        return out

    ts1 = _self_arr(t1n)
    inB = [{"tab": _build_table(plan, t1n), "idx": plan["idxw"][d],
            "dinvw": plan["dinvw"][d], "b1": b1[:, None].astype(np.float32),
            "tself": ts1[d], "sel": sel,
            "ident": np.eye(HID, dtype=np.float16)}
           for d in range(NDEV)]
    resB = _run(ncB, inB)
    t2n = np.zeros((N, HID), np.float16)
    for d in range(NDEV):
        m = nor[ridx[d]] >= 0
        t2n[nor[ridx[d]][m]] = resB[d]["t2"][m]

    selw2 = np.zeros((128, NCLS, DOUT), np.float32)
    for s in range(NCLS):
        selw2[16 * s + np.arange(HID), s, :] = W2
    ws = _self_arr((t2n.astype(np.float32) @ W2).astype(np.float16))
    inC = [{"tab": _build_table(plan, t2n), "idx": plan["idxw"][d],
            "dinvw": plan["dinvw"][d], "b2": b2[:, None].astype(np.float32),
            "wself": ws[d], "selw2": selw2} for d in range(NDEV)]
    resC = _run(ncC, inC)
    out = np.zeros((N, DOUT), np.float32)
    for d in range(NDEV):
        m = nor[ridx[d]] >= 0
        out[nor[ridx[d]][m]] = resC[d]["o2"].T[m]
    return out



# revision 13
# speedup vs baseline: 1.2015x; 1.0020x over previous
"""Trainium2 Bass kernel for 2-layer GCN (GCNConv -> ReLU -> GCNConv).

v2 strategy — SBUF-resident fp16 tables + transpose-mode SBUF-source gathers
(the baseline's HBM dma_gather was HBM-latency bound at ~63ns/edge):

- Both layers reduce to: gather 16-wide rows t[src], segment-sum by dst
  (linear layers commute with the normalized aggregation).
- The 16-fp16 (32B) node records live in SBUF as [128 part, W windows] of
  256B chunks; chunk (tok, w) holds 8 records at positions s=0..7.
- A token (edge) gathers its source's 256B chunk via dma_gather(transpose=
  True, SBUF source): the chunk becomes a 128-partition fp16 column; the
  wanted record sits at partition slice [16s, 16s+16) where s = the record's
  chunk position ("class").  Chunk-mates land on other slices — never read.
- Host assigns each node TWO candidate classes and each edge picks one
  (power-of-two-choices), balancing per-(destination-group, class) slot
  counts K.  Grid columns per group g: [class s][slot k<K[g,s]][win j][dst p]
  so one strided DVE tensor_reduce per (group, class) segment-sums slot
  layers into slice s of a [128, 256] tile P.  P's 8 slices collapse via a
  PE matmul with a 0/1 selector (layer 2 fuses W2 into the selector).
- 3 SPMD launches: A (t1 = dinv*x@W1), B (layer-1 aggregate -> t2),
  C (layer-2 aggregate -> @W2+b2).  Host re-shards tables between launches.
"""
import os
import sys

sys.path.insert(0, "/opt/trn_rl_repo")

import numpy as np

import concourse.bass as bass
import concourse.mybir as mybir
import concourse.tile as tile
from concourse import bacc, bass_utils, library_config

N = 100000
E = 1600000
DIN, HID, DOUT = 256, 16, 64
NDEV = 8
NCLS = 8
GW = 1                      # windows per K-uniform group
F32 = mybir.dt.float32
F16 = mybir.dt.float16
I16 = mybir.dt.int16
NQUEUES = int(os.environ.get("GCN_NQUEUES", "1"))
GCHUNK = int(os.environ.get("GCN_GCHUNK", "2944"))
DMASCRATCH = int(os.environ.get("GCN_DMASCRATCH", "49152"))
N3 = int(os.environ.get("GCN_N3", "62000"))
MAXSLOTS = 32640              # 255 windows/class keeps idx_pad in int16

LAST_EXEC_NS = []


# ----------------------------------------------------------------------------
# host-side graph planning
# ----------------------------------------------------------------------------

def _ragged_arange(lens):
    ends = np.cumsum(lens)
    total = int(ends[-1]) if len(lens) else 0
    out = np.arange(total, dtype=np.int64)
    out -= np.repeat(ends - lens, lens)
    return out


def _plan(edge_index, seed=12345):
    rng = np.random.default_rng(seed)
    src = np.asarray(edge_index[0], dtype=np.int64)
    dst = np.asarray(edge_index[1], dtype=np.int64)
    # self-loops are NOT tokens: the dst's own record is added in postproc
    all_src = src
    all_dst = dst
    T = len(all_src)
    indeg = np.bincount(dst, minlength=N).astype(np.int64) + 1  # GCN degree
    dinv_n = (1.0 / np.sqrt(indeg.astype(np.float64))).astype(np.float32)

    # rank deal: degree-sorted; i-th -> device i%8, window (i//8)//128
    order = np.argsort(-indeg, kind="stable")
    di = np.empty(N, np.int64)
    di[order] = np.arange(N)
    dev_n = di % NDEV
    w_n = (di // NDEV) // 128
    p_n = (di // NDEV) % 128
    nwin = int(w_n.max()) + 1
    ngrp = (nwin + GW - 1) // GW
    nwin_pad = ngrp * GW
    rank_n = (w_n * NDEV + dev_n) * 128 + p_n
    nloc = nwin_pad * 128
    npad = nloc * NDEV

    grp_n = w_n // GW

    # --- class assignment: mixed-R candidates + balanced greedy + repair ---
    ko = np.argsort(all_dst, kind="stable")
    t_dst = all_dst[ko]
    t_src = all_src[ko]
    tg = grp_n[t_dst]

    def _make_cand(n3):
        s1 = rng.integers(0, NCLS, N)
        s2 = (s1 + 1 + rng.integers(0, NCLS - 1, N)) % NCLS
        cand = np.stack([s1, s2], 1)
        if n3 > 0:
            odeg = np.bincount(all_src, minlength=N)
            top = np.argsort(-odeg, kind="stable")[:n3]
            s3 = rng.integers(0, NCLS, n3)
            bad = (s3 == s1[top]) | (s3 == s2[top])
            while bad.any():
                s3[bad] = rng.integers(0, NCLS, bad.sum())
                bad = (s3 == s1[top]) | (s3 == s2[top])
            c3 = np.full(N, -1, np.int64)
            c3[top] = s3
            cand = np.concatenate([cand, c3[:, None]], 1)
        return cand

    starts = np.searchsorted(t_dst, np.arange(N + 1))
    pos = np.arange(T) - np.repeat(starts[:-1], np.diff(starts))

    def _greedy(cand):
        cnt = np.zeros((N, NCLS), np.int16)
        cls_tok = np.zeros(T, np.int8)
        for k in range(int(pos.max()) + 1):
            m = np.flatnonzero(pos == k)
            if len(m) == 0:
                break
            u, v = t_src[m], t_dst[m]
            cc = cand[u]
            counts = np.where(cc >= 0, cnt[v[:, None], np.maximum(cc, 0)], 127)
            best = np.argmin(counts, axis=1)
            c = cc[np.arange(len(m)), best]
            cls_tok[m] = c
            cnt[v, c] += 1
        return cls_tok, cnt

    def _repair(cls_tok, cnt, cand, iters=400):
        R = cand.shape[1]
        cls_tok = cls_tok.astype(np.int64)
        tc = cand[t_src]
        for it in range(iters):
            K = np.zeros((ngrp, NCLS), np.int64)
            np.maximum.at(K, (tg, cls_tok), cnt[t_dst, cls_tok])
            K = np.maximum(K, 1)
            cur = cls_tok
            crit = cnt[t_dst, cur] == K[tg, cur]
            alt_ok = np.full(T, -1, np.int64)
            for r in rng.permutation(R):
                a = tc[:, r]
                a0 = np.maximum(a, 0)
                ok = (crit & (a >= 0) & (a != cur) & (alt_ok < 0)
                      & (cnt[t_dst, a0] + 1 < K[tg, a0]))
                alt_ok[ok] = a[ok]
            mv = np.flatnonzero(alt_ok >= 0)
            if len(mv) == 0:
                # 2-chain: evict a non-critical blocker from (dst, a) where a
                # sits one below the group max, freeing room for a critical
                # token to move there next iteration.
                relief = np.full(T, -1, np.int64)
                for r in rng.permutation(R):
                    a = tc[:, r]
                    a0 = np.maximum(a, 0)
                    ok = (crit & (a >= 0) & (a != cur) & (relief < 0)
                          & (cnt[t_dst, a0] + 1 == K[tg, a0])
                          & (cnt[t_dst, a0] > 0))
                    relief[ok] = a[ok]
                want = np.flatnonzero(relief >= 0)
                if len(want) == 0:
                    break
                wantset = np.zeros(N * NCLS, bool)
                wantset[t_dst[want] * NCLS + relief[want]] = True
                isblk = wantset[t_dst * NCLS + cur] & ~crit
                blk_alt = np.full(T, -1, np.int64)
                for r in rng.permutation(R):
                    a = tc[:, r]
                    a0 = np.maximum(a, 0)
                    ok = (isblk & (a >= 0) & (a != cur) & (blk_alt < 0)
                          & (cnt[t_dst, a0] + 1 < K[tg, a0]))
                    blk_alt[ok] = a[ok]
                mv = np.flatnonzero(blk_alt >= 0)
                if len(mv) == 0:
                    break
                alt_ok = blk_alt
            ks = np.argsort(t_dst[mv], kind="stable")
            kk = t_dst[mv][ks]
            first = np.concatenate([[True], kk[1:] != kk[:-1]])
            mv = mv[ks[first]]
            cv, av, vv = cur[mv].copy(), alt_ok[mv], t_dst[mv]
            cls_tok[mv] = av
            np.subtract.at(cnt, (vv, cv), 1)
            np.add.at(cnt, (vv, av), 1)
        K = np.zeros((ngrp, NCLS), np.int64)
        np.maximum.at(K, (tg, cls_tok), cnt[t_dst, cls_tok])
        return cls_tok.astype(np.int8), cnt, np.maximum(K, 1)

    n3 = N3
    while True:
        cand = _make_cand(n3)
        cls_tok, cnt = _greedy(cand)
        cls_tok, cnt, K = _repair(cls_tok, cnt, cand)
        used_chk = np.zeros((N, NCLS), bool)
        used_chk[t_src, cls_tok.astype(np.int64)] = True
        if used_chk.sum(0).max() <= MAXSLOTS or n3 == 0:
            break
        n3 = max(0, n3 - 15000)

    # --- table slot allocation per class ---
    used = np.zeros((N, NCLS), bool)
    used[t_src, cls_tok.astype(np.int64)] = True
    gidx = np.zeros((N, NCLS), np.int32)
    slot_u, slot_s, slot_q = [], [], []
    nwt = 0
    for s in range(NCLS):
        us = np.flatnonzero(used[:, s])
        q = np.arange(len(us))
        gidx[us, s] = (q // 128) * 128 + (q % 128)
        slot_u.append(us)
        slot_s.append(np.full(len(us), s))
        slot_q.append(q)
        nwt = max(nwt, (len(us) + 127) // 128)
    slot_u = np.concatenate(slot_u)
    slot_s = np.concatenate(slot_s)
    slot_q = np.concatenate(slot_q)
    idx_pad = nwt * 128                    # zero window
    assert idx_pad + 127 < 32768

    # --- grid column layout (global K; identical on all devices) ---
    offs = np.concatenate([np.zeros((ngrp, 1), np.int64),
                           np.cumsum(K, axis=1)], axis=1) * (GW * 128)
    Cg = offs[:, -1]
    grpbase = np.concatenate([[0], np.cumsum(Cg)])
    T_dev = int(grpbase[-1])
    assert T_dev % 16 == 0

    # gather split per group: class boundary nearest the middle
    split = []
    for g in range(ngrp):
        sh = int(np.argmin(np.abs(offs[g, 1:-1] - Cg[g] / 2))) + 1
        split.append((sh, int(offs[g, sh])))

    # --- token -> column, idx arrays per device ---
    occ = np.empty(T, np.int64)
    key2 = t_dst * NCLS + cls_tok.astype(np.int64)
    k2o = np.argsort(key2, kind="stable")
    kk2 = key2[k2o]
    bnd = np.concatenate([[True], kk2[1:] != kk2[:-1]])
    gstarts = np.flatnonzero(bnd)
    glens = np.diff(np.concatenate([gstarts, [T]]))
    occ[k2o] = _ragged_arange(glens)

    v = t_dst
    g = grp_n[v]
    col = (grpbase[g] + offs[g, cls_tok.astype(np.int64)]
           + occ * (GW * 128) + (w_n[v] % GW) * 128 + p_n[v])
    tdev = dev_n[v]
    srcval = gidx[t_src, cls_tok.astype(np.int64)].astype(np.int16)
    idxw = np.empty((NDEV, 128, T_dev // 16), np.int16)
    for d in range(NDEV):
        m = tdev == d
        a = np.full(T_dev, idx_pad, np.int16)
        a[col[m]] = srcval[m]
        idxw[d] = np.tile(a.reshape(T_dev // 16, 16).T, (8, 1))

    # --- per-device aux arrays ---
    ridx = np.empty((NDEV, nloc), np.int64)
    for d in range(NDEV):
        gg = ((np.arange(nwin_pad) * NDEV + d)[:, None] * 128 + np.arange(128))
        ridx[d] = gg.reshape(-1)
    node_of_rank = np.full(npad, -1, np.int64)
    node_of_rank[rank_n] = np.arange(N)
    dinv_r = np.zeros(npad, np.float32)
    dinv_r[rank_n] = dinv_n

    dinva = np.empty((NDEV, 128, nwin_pad), np.float32)
    dinvw = np.empty((NDEV, 64, nloc), np.float16)
    for d in range(NDEV):
        dr = dinv_r[ridx[d]]
        dinva[d] = dr.reshape(nwin_pad, 128).T
        dinvw[d] = np.tile(dr[None, :], (64, 1)).astype(np.float16)

    return dict(
        nwin=nwin_pad, ngrp=ngrp, nloc=nloc, npad=npad, nwt=nwt,
        idx_pad=idx_pad, K=K, offs=offs, Cg=Cg, grpbase=grpbase,
        T_dev=T_dev, split=split, idxw=idxw, ridx=ridx,
        node_of_rank=node_of_rank, rank_n=rank_n, dinva=dinva, dinvw=dinvw,
        slot_u=slot_u, slot_s=slot_s, slot_q=slot_q,
    )


# ----------------------------------------------------------------------------
# device programs
# ----------------------------------------------------------------------------

def _build_A(plan):
    nwin, nloc = plan["nwin"], plan["nloc"]
    nc = bacc.Bacc("TRN2", target_bir_lowering=False, debug=False,
                   num_devices=NDEV)
    xT_d = nc.dram_tensor("xT", [DIN, nloc], F32, kind="ExternalInput").ap()
    dinva_d = nc.dram_tensor("dinva", [128, nwin], F32,
                             kind="ExternalInput").ap()
    w1_d = nc.dram_tensor("w1", [128, 2, HID], F32, kind="ExternalInput").ap()
    t1_d = nc.dram_tensor("t1", [nloc, HID], F16, kind="ExternalOutput").ap()

    with tile.TileContext(nc) as tc:
        with (
            tc.tile_pool(name="cst", bufs=1) as cst,
            tc.tile_pool(name="xp", bufs=3) as xp,
            tc.tile_pool(name="ps", bufs=2, space="PSUM") as psp,
            tc.tile_pool(name="stg", bufs=2) as stg,
        ):
            w1t = cst.tile([128, 2, HID], F32)
            nc.sync.dma_start(out=w1t[:], in_=w1_d[:])
            dat = cst.tile([128, nwin], F32)
            nc.sync.dma_start(out=dat[:], in_=dinva_d[:])
            ov = t1_d.rearrange("(w p) f -> p w f", p=128)
            for i0 in range(0, nwin, 8):
                nb = min(8, nwin - i0)
                xts = []
                for k in range(2):
                    xt = xp.tile([128, 8 * 128], F32, tag=f"xt{k}")
                    nc.sync.dma_start(
                        out=xt[:, :nb * 128],
                        in_=xT_d[k * 128:(k + 1) * 128,
                                 i0 * 128:(i0 + nb) * 128],
                    )
                    xts.append(xt)
                stage = stg.tile([128, 8, HID], F16)
                for ib in range(nb):
                    ps = psp.tile([128, HID], F32)
                    for k in range(2):
                        nc.tensor.matmul(
                            out=ps[:],
                            lhsT=xts[k][:, ib * 128:(ib + 1) * 128],
                            rhs=w1t[:, k, :],
                            start=(k == 0), stop=(k == 1),
                        )
                    nc.vector.tensor_scalar_mul(
                        out=stage[:, ib, :], in0=ps[:],
                        scalar1=dat[:, i0 + ib:i0 + ib + 1],
                    )
                nc.sync.dma_start(out=ov[:, i0:i0 + nb, :],
                                  in_=stage[:, :nb, :])
    nc.compile()
    return nc


def _sbuf_gather(nc, out_sl, tabt, idx_sl, ncols, q):
    nc.gpsimd.dma_gather(
        out_sl, tabt[:], idx_sl,
        num_idxs=ncols, num_idxs_reg=ncols, elem_size=128,
        transpose=True, single_packet=False, queue_num=q,
        sbuf_tokens_per_rank=128, sbuf_free_dim_per_rank=256,
        sbuf_free_dim_pad_per_rank=0, sbuf_byte_offset=0,
    )


def _reduce_cls(nc, Ps, vt, c0, Kgs):
    """Full-128-partition strided segment-sum of one class's token columns.
    Only partitions [16s, 16s+16) of the result are meaningful; the selector
    matmul extracts them (DVE cost is free-size driven, partitions are
    parallel lanes, and 16-partition slices at odd 16-offsets violate the
    32-alignment rule)."""
    sl = vt[:, 0, c0:c0 + GW * 128 * Kgs]
    rap = bass.AP(
        sl.tensor, sl.offset,
        [list(sl.ap[0]), [1, GW * 128], [GW * 128, Kgs]],
    )
    nc.vector.tensor_reduce(
        out=Ps[:], in_=rap,
        axis=mybir.AxisListType.X, op=mybir.AluOpType.add,
    )


def _build_BC(plan, layer):
    nwin, ngrp, nloc, nwt = (plan["nwin"], plan["ngrp"], plan["nloc"],
                             plan["nwt"])
    K, offs, Cg, grpbase, split, T_dev = (plan["K"], plan["offs"], plan["Cg"],
                                          plan["grpbase"], plan["split"],
                                          plan["T_dev"])
    WB = GW * 128
    cg_max = max(int(c) for c in Cg)

    nc = bacc.Bacc("TRN2", target_bir_lowering=False, debug=False,
                   num_devices=NDEV, num_swdge_queues=NQUEUES,
                   dynamic_dma_scratch_size=DMASCRATCH)
    tab_d = nc.dram_tensor("tab", [128, (nwt + 1) * 128], F16,
                           kind="ExternalInput").ap()
    idx_d = nc.dram_tensor("idx", [128, T_dev // 16], I16,
                           kind="ExternalInput").ap()
    dinvw_d = nc.dram_tensor("dinvw", [64, nloc], F16,
                             kind="ExternalInput").ap()
    if layer == 1:
        b_d = nc.dram_tensor("b1", [HID, 1], F32, kind="ExternalInput").ap()
        ts_d = nc.dram_tensor("tself", [HID, nloc], F16,
                              kind="ExternalInput").ap()
        sel_d = nc.dram_tensor("sel", [128, NCLS, HID], F32,
                               kind="ExternalInput").ap()
        id_d = nc.dram_tensor("ident", [HID, HID], F16,
                              kind="ExternalInput").ap()
        o_d = nc.dram_tensor("t2", [nloc, HID], F16,
                             kind="ExternalOutput").ap()
        ov = o_d.rearrange("(w p) f -> p w f", p=128)
    else:
        b_d = nc.dram_tensor("b2", [DOUT, 1], F32, kind="ExternalInput").ap()
        ts_d = nc.dram_tensor("wself", [DOUT, nloc], F16,
                              kind="ExternalInput").ap()
        sel_d = nc.dram_tensor("selw2", [128, NCLS, DOUT], F32,
                               kind="ExternalInput").ap()
        o_d = nc.dram_tensor("o2", [DOUT, nloc], F32,
                             kind="ExternalOutput").ap()
        ov = o_d

    with tile.TileContext(nc) as tc:
        with (
            tc.tile_pool(name="cst", bufs=1) as cst,
            tc.tile_pool(name="ip", bufs=3) as ip,
            tc.tile_pool(name="vp", bufs=2) as vp,
            tc.tile_pool(name="pp", bufs=3) as pp,
            tc.tile_pool(name="sm", bufs=3) as sm,
            tc.tile_pool(name="pY", bufs=2, space="PSUM") as pY,
            tc.tile_pool(name="pT", bufs=2, space="PSUM") as pT,
            tc.tile_pool(name="stg", bufs=2) as stg,
        ):
            nc.gpsimd.load_library(library_config.mlp)
            tabt = cst.tile([128, (nwt + 1) * 128], F16)
            nc.sync.dma_start(out=tabt[:], in_=tab_d[:])
            dvw = cst.tile([64, nloc], F16)
            nc.sync.dma_start(out=dvw[:], in_=dinvw_d[:])
            nb_ = HID if layer == 1 else DOUT
            bt = cst.tile([nb_, 1], F32)
            nc.sync.dma_start(out=bt[:], in_=b_d[:])
            tsl = cst.tile([nb_, nloc], F16)
            nc.sync.dma_start(out=tsl[:], in_=ts_d[:])
            selt = cst.tile([128, NCLS, nb_], F32)
            nc.sync.dma_start(out=selt[:], in_=sel_d[:])
            if layer == 1:
                idt = cst.tile([HID, HID], F16)
                nc.sync.dma_start(out=idt[:], in_=id_d[:])

            it_max = cg_max // 16
            qctr = 0
            for g in range(ngrp):
                cg = int(Cg[g])
                t0 = int(grpbase[g])
                it = ip.tile([128, it_max], I16, tag="idx")
                nc.sync.dma_start(out=it[:, :cg // 16],
                                  in_=idx_d[:, t0 // 16:(t0 + cg) // 16])
                va = vp.tile([128, 1, cg_max], F16, tag="va")
                # NOTE: concurrent gathers on rotated SWDGE queues (NQUEUES>1)
                # are 3.5x faster but corrupt data: the per-queue transpose
                # streams interleave packet-wise on the shared XBAR. Keep
                # NQUEUES=1; chunk size amortizes per-instruction overhead
                # within the scratch ring (DMASCRATCH/16 descs per side).
                nch = (cg + GCHUNK - 1) // GCHUNK
                bnds = [(cg * i // nch) // 128 * 128 for i in range(nch + 1)]
                bnds[-1] = cg
                for c0, c1 in zip(bnds[:-1], bnds[1:]):
                    _sbuf_gather(nc, va[:, :, c0:c1], tabt,
                                 it[:, c0 // 16:c1 // 16], c1 - c0,
                                 qctr % NQUEUES)
                    qctr += 1
                Y = pY.tile([nb_, WB], F32)
                for s in range(NCLS):
                    Kgs = int(K[g, s])
                    Ps = pp.tile([128, WB], F32, tag=f"P{s}")
                    _reduce_cls(nc, Ps, va, int(offs[g, s]), Kgs)
                    nc.tensor.matmul(out=Y[:], lhsT=selt[:, s, :], rhs=Ps[:],
                                     start=(s == 0), stop=(s == NCLS - 1))
                dsl = dvw[0:nb_, g * WB:(g + 1) * WB]
                ya = sm.tile([nb_, WB], F32, tag="ya")
                nc.vector.scalar_tensor_tensor(
                    out=ya[:], in0=Y[:], scalar=1.0,
                    in1=tsl[:, g * WB:(g + 1) * WB],
                    op0=mybir.AluOpType.mult, op1=mybir.AluOpType.add,
                )
                if layer == 1:
                    yd = sm.tile([HID, WB], F32, tag="yd")
                    nc.vector.scalar_tensor_tensor(
                        out=yd[:], in0=ya[:], scalar=1.0, in1=dsl,
                        op0=mybir.AluOpType.mult, op1=mybir.AluOpType.mult,
                    )
                    r = sm.tile([HID, WB], F32, tag="r")
                    nc.scalar.activation(
                        out=r[:], in_=yd[:],
                        func=mybir.ActivationFunctionType.Relu,
                        bias=bt[:, 0:1],
                    )
                    t2c = sm.tile([HID, WB], F16, tag="t2c")
                    nc.vector.scalar_tensor_tensor(
                        out=t2c[:], in0=r[:], scalar=1.0, in1=dsl,
                        op0=mybir.AluOpType.mult, op1=mybir.AluOpType.mult,
                    )
                    stage = stg.tile([128, GW, HID], F16)
                    for j in range(GW):
                        tp = pT.tile([128, HID], F16)
                        nc.tensor.transpose(
                            out=tp[:], in_=t2c[:, j * 128:(j + 1) * 128],
                            identity=idt[:])
                        nc.vector.tensor_copy(out=stage[:, j, :], in_=tp[:])
                    nc.sync.dma_start(out=ov[:, g * GW:(g + 1) * GW, :],
                                      in_=stage[:])
                else:
                    od = sm.tile([DOUT, WB], F32, tag="od")
                    nc.vector.scalar_tensor_tensor(
                        out=od[:], in0=ya[:], scalar=1.0, in1=dsl,
                        op0=mybir.AluOpType.mult, op1=mybir.AluOpType.mult,
                    )
                    stage = stg.tile([DOUT, WB], F32)
                    nc.vector.tensor_scalar_add(
                        out=stage[:], in0=od[:], scalar1=bt[:, 0:1],
                    )
                    nc.sync.dma_start(out=ov[:, g * WB:(g + 1) * WB],
                                      in_=stage[:])
    nc.compile()
    return nc


# ----------------------------------------------------------------------------
# driver
# ----------------------------------------------------------------------------

_PROG_CACHE = {}


def _run(nc, in_maps):
    trace = os.environ.get("GCN_TRACE", "0") == "1"
    res = bass_utils.run_bass_kernel_spmd(
        nc, in_maps, core_ids=list(range(NDEV)), trace=trace)
    if res.exec_time_ns is not None:
        LAST_EXEC_NS.append(int(res.exec_time_ns))
    return res.results


def _build_table(plan, t_node):
    nwt = plan["nwt"]
    tab = np.zeros((128, nwt + 1, NCLS, HID), np.float16)
    q, s, u = plan["slot_q"], plan["slot_s"], plan["slot_u"]
    tab[q % 128, q // 128, s] = t_node[u]
    return np.ascontiguousarray(tab.reshape(128, (nwt + 1) * 128))


def kernel(x, edge_index, W1, b1, W2, b2):
    LAST_EXEC_NS.clear()
    x = np.asarray(x, np.float32)
    W1 = np.asarray(W1, np.float32)
    b1 = np.asarray(b1, np.float32)
    W2 = np.asarray(W2, np.float32)
    b2 = np.asarray(b2, np.float32)

    ei = np.asarray(edge_index)
    plan = _plan(ei, seed=99)
    key = ("v6", plan["nwin"], plan["T_dev"], int(plan["K"].sum()))
    if key not in _PROG_CACHE:
        _PROG_CACHE.clear()
        _PROG_CACHE[key] = (_build_A(plan), _build_BC(plan, 1),
                            _build_BC(plan, 2))
    ncA, ncB, ncC = _PROG_CACHE[key]

    ridx, nor = plan["ridx"], plan["node_of_rank"]
    npad, nloc = plan["npad"], plan["nloc"]

    xfull = np.zeros((npad, DIN), np.float32)
    xfull[plan["rank_n"]] = x
    w1r = np.ascontiguousarray(
        W1.reshape(2, 128, HID).transpose(1, 0, 2))
    inA = [{"xT": np.ascontiguousarray(xfull[ridx[d]].T),
            "dinva": plan["dinva"][d], "w1": w1r} for d in range(NDEV)]
    resA = _run(ncA, inA)
    t1n = np.zeros((N, HID), np.float16)
    for d in range(NDEV):
        m = nor[ridx[d]] >= 0
        t1n[nor[ridx[d]][m]] = resA[d]["t1"][m]

    sel = np.zeros((128, NCLS, HID), np.float32)
    for s in range(NCLS):
        sel[16 * s + np.arange(HID), s, np.arange(HID)] = 1.0
    def _self_arr(tn):
        out = []
        for d in range(NDEV):
            a = np.zeros((nloc, tn.shape[1]), np.float16)
            m = nor[ridx[d]] >= 0
            a[m] = tn[nor[ridx[d]][m]]
            out.append(np.ascontiguousarray(a.T))
        return out

    ts1 = _self_arr(t1n)
    inB = [{"tab": _build_table(plan, t1n), "idx": plan["idxw"][d],
            "dinvw": plan["dinvw"][d], "b1": b1[:, None].astype(np.float32),
            "tself": ts1[d], "sel": sel,
            "ident": np.eye(HID, dtype=np.float16)}
           for d in range(NDEV)]
    resB = _run(ncB, inB)
    t2n = np.zeros((N, HID), np.float16)
    for d in range(NDEV):
        m = nor[ridx[d]] >= 0
        t2n[nor[ridx[d]][m]] = resB[d]["t2"][m]

    selw2 = np.zeros((128, NCLS, DOUT), np.float32)
    for s in range(NCLS):
        selw2[16 * s + np.arange(HID), s, :] = W2
    ws = _self_arr((t2n.astype(np.float32) @ W2).astype(np.float16))
    inC = [{"tab": _build_table(plan, t2n), "idx": plan["idxw"][d],
            "dinvw": plan["dinvw"][d], "b2": b2[:, None].astype(np.float32),
            "wself": ws[d], "selw2": selw2} for d in range(NDEV)]
    resC = _run(ncC, inC)
    out = np.zeros((N, DOUT), np.float32)
    for d in range(NDEV):
        m = nor[ridx[d]] >= 0
        out[nor[ridx[d]][m]] = resC[d]["o2"].T[m]
    return out



# revision 15
# speedup vs baseline: 1.2133x; 1.0098x over previous
"""Trainium2 Bass kernel for 2-layer GCN (GCNConv -> ReLU -> GCNConv).

v2 strategy — SBUF-resident fp16 tables + transpose-mode SBUF-source gathers
(the baseline's HBM dma_gather was HBM-latency bound at ~63ns/edge):

- Both layers reduce to: gather 16-wide rows t[src], segment-sum by dst
  (linear layers commute with the normalized aggregation).
- The 16-fp16 (32B) node records live in SBUF as [128 part, W windows] of
  256B chunks; chunk (tok, w) holds 8 records at positions s=0..7.
- A token (edge) gathers its source's 256B chunk via dma_gather(transpose=
  True, SBUF source): the chunk becomes a 128-partition fp16 column; the
  wanted record sits at partition slice [16s, 16s+16) where s = the record's
  chunk position ("class").  Chunk-mates land on other slices — never read.
- Host assigns each node TWO candidate classes and each edge picks one
  (power-of-two-choices), balancing per-(destination-group, class) slot
  counts K.  Grid columns per group g: [class s][slot k<K[g,s]][win j][dst p]
  so one strided DVE tensor_reduce per (group, class) segment-sums slot
  layers into slice s of a [128, 256] tile P.  P's 8 slices collapse via a
  PE matmul with a 0/1 selector (layer 2 fuses W2 into the selector).
- 3 SPMD launches: A (t1 = dinv*x@W1), B (layer-1 aggregate -> t2),
  C (layer-2 aggregate -> @W2+b2).  Host re-shards tables between launches.
"""
import os
import sys

sys.path.insert(0, "/opt/trn_rl_repo")

import numpy as np

import concourse.bass as bass
import concourse.mybir as mybir
import concourse.tile as tile
from concourse import bacc, bass_utils, library_config

N = 100000
E = 1600000
DIN, HID, DOUT = 256, 16, 64
NDEV = 8
NCLS = 8
GW = 1                      # windows per K-uniform group
F32 = mybir.dt.float32
F16 = mybir.dt.float16
I16 = mybir.dt.int16
NQUEUES = int(os.environ.get("GCN_NQUEUES", "1"))
GCHUNK = int(os.environ.get("GCN_GCHUNK", "2944"))
DMASCRATCH = int(os.environ.get("GCN_DMASCRATCH", "49152"))
N3 = int(os.environ.get("GCN_N3", "62000"))
MAXSLOTS = 32640              # 255 windows/class keeps idx_pad in int16

LAST_EXEC_NS = []


# ----------------------------------------------------------------------------
# host-side graph planning
# ----------------------------------------------------------------------------

def _ragged_arange(lens):
    ends = np.cumsum(lens)
    total = int(ends[-1]) if len(lens) else 0
    out = np.arange(total, dtype=np.int64)
    out -= np.repeat(ends - lens, lens)
    return out


def _plan(edge_index, seed=12345):
    rng = np.random.default_rng(seed)
    src = np.asarray(edge_index[0], dtype=np.int64)
    dst = np.asarray(edge_index[1], dtype=np.int64)
    # self-loops are NOT tokens: the dst's own record is added in postproc
    all_src = src
    all_dst = dst
    T = len(all_src)
    indeg = np.bincount(dst, minlength=N).astype(np.int64) + 1  # GCN degree
    dinv_n = (1.0 / np.sqrt(indeg.astype(np.float64))).astype(np.float32)

    # rank deal: degree-sorted; i-th -> device i%8, window (i//8)//128
    order = np.argsort(-indeg, kind="stable")
    di = np.empty(N, np.int64)
    di[order] = np.arange(N)
    dev_n = di % NDEV
    w_n = (di // NDEV) // 128
    p_n = (di // NDEV) % 128
    nwin = int(w_n.max()) + 1
    ngrp = (nwin + GW - 1) // GW
    nwin_pad = ngrp * GW
    rank_n = (w_n * NDEV + dev_n) * 128 + p_n
    nloc = nwin_pad * 128
    npad = nloc * NDEV

    grp_n = w_n // GW

    # --- class assignment: mixed-R candidates + balanced greedy + repair ---
    ko = np.argsort(all_dst, kind="stable")
    t_dst = all_dst[ko]
    t_src = all_src[ko]
    tg = grp_n[t_dst]

    def _make_cand(n3):
        s1 = rng.integers(0, NCLS, N)
        s2 = (s1 + 1 + rng.integers(0, NCLS - 1, N)) % NCLS
        cand = np.stack([s1, s2], 1)
        if n3 > 0:
            odeg = np.bincount(all_src, minlength=N)
            top = np.argsort(-odeg, kind="stable")[:n3]
            s3 = rng.integers(0, NCLS, n3)
            bad = (s3 == s1[top]) | (s3 == s2[top])
            while bad.any():
                s3[bad] = rng.integers(0, NCLS, bad.sum())
                bad = (s3 == s1[top]) | (s3 == s2[top])
            c3 = np.full(N, -1, np.int64)
            c3[top] = s3
            cand = np.concatenate([cand, c3[:, None]], 1)
        return cand

    starts = np.searchsorted(t_dst, np.arange(N + 1))
    pos = np.arange(T) - np.repeat(starts[:-1], np.diff(starts))

    def _greedy(cand):
        cnt = np.zeros((N, NCLS), np.int16)
        cls_tok = np.zeros(T, np.int8)
        for k in range(int(pos.max()) + 1):
            m = np.flatnonzero(pos == k)
            if len(m) == 0:
                break
            u, v = t_src[m], t_dst[m]
            cc = cand[u]
            counts = np.where(cc >= 0, cnt[v[:, None], np.maximum(cc, 0)], 127)
            best = np.argmin(counts, axis=1)
            c = cc[np.arange(len(m)), best]
            cls_tok[m] = c
            cnt[v, c] += 1
        return cls_tok, cnt

    def _repair(cls_tok, cnt, cand, iters=400):
        R = cand.shape[1]
        cls_tok = cls_tok.astype(np.int64)
        tc = cand[t_src]
        for it in range(iters):
            K = np.zeros((ngrp, NCLS), np.int64)
            np.maximum.at(K, (tg, cls_tok), cnt[t_dst, cls_tok])
            K = np.maximum(K, 1)
            cur = cls_tok
            crit = cnt[t_dst, cur] == K[tg, cur]
            alt_ok = np.full(T, -1, np.int64)
            for r in rng.permutation(R):
                a = tc[:, r]
                a0 = np.maximum(a, 0)
                ok = (crit & (a >= 0) & (a != cur) & (alt_ok < 0)
                      & (cnt[t_dst, a0] + 1 < K[tg, a0]))
                alt_ok[ok] = a[ok]
            mv = np.flatnonzero(alt_ok >= 0)
            if len(mv) == 0:
                # 2-chain: evict a non-critical blocker from (dst, a) where a
                # sits one below the group max, freeing room for a critical
                # token to move there next iteration.
                relief = np.full(T, -1, np.int64)
                for r in rng.permutation(R):
                    a = tc[:, r]
                    a0 = np.maximum(a, 0)
                    ok = (crit & (a >= 0) & (a != cur) & (relief < 0)
                          & (cnt[t_dst, a0] + 1 == K[tg, a0])
                          & (cnt[t_dst, a0] > 0))
                    relief[ok] = a[ok]
                want = np.flatnonzero(relief >= 0)
                if len(want) == 0:
                    break
                wantset = np.zeros(N * NCLS, bool)
                wantset[t_dst[want] * NCLS + relief[want]] = True
                isblk = wantset[t_dst * NCLS + cur] & ~crit
                blk_alt = np.full(T, -1, np.int64)
                for r in rng.permutation(R):
                    a = tc[:, r]
                    a0 = np.maximum(a, 0)
                    ok = (isblk & (a >= 0) & (a != cur) & (blk_alt < 0)
                          & (cnt[t_dst, a0] + 1 < K[tg, a0]))
                    blk_alt[ok] = a[ok]
                mv = np.flatnonzero(blk_alt >= 0)
                if len(mv) == 0:
                    break
                alt_ok = blk_alt
            ks = np.argsort(t_dst[mv], kind="stable")
            kk = t_dst[mv][ks]
            first = np.concatenate([[True], kk[1:] != kk[:-1]])
            mv = mv[ks[first]]
            cv, av, vv = cur[mv].copy(), alt_ok[mv], t_dst[mv]
            cls_tok[mv] = av
            np.subtract.at(cnt, (vv, cv), 1)
            np.add.at(cnt, (vv, av), 1)
        K = np.zeros((ngrp, NCLS), np.int64)
        np.maximum.at(K, (tg, cls_tok), cnt[t_dst, cls_tok])
        return cls_tok.astype(np.int8), cnt, np.maximum(K, 1)

    n3 = N3
    while True:
        cand = _make_cand(n3)
        cls_tok, cnt = _greedy(cand)
        cls_tok, cnt, K = _repair(cls_tok, cnt, cand)
        used_chk = np.zeros((N, NCLS), bool)
        used_chk[t_src, cls_tok.astype(np.int64)] = True
        if used_chk.sum(0).max() <= MAXSLOTS or n3 == 0:
            break
        n3 = max(0, n3 - 15000)

    # --- table slot allocation per class ---
    used = np.zeros((N, NCLS), bool)
    used[t_src, cls_tok.astype(np.int64)] = True
    gidx = np.zeros((N, NCLS), np.int32)
    slot_u, slot_s, slot_q = [], [], []
    nwt = 0
    for s in range(NCLS):
        us = np.flatnonzero(used[:, s])
        q = np.arange(len(us))
        gidx[us, s] = (q // 128) * 128 + (q % 128)
        slot_u.append(us)
        slot_s.append(np.full(len(us), s))
        slot_q.append(q)
        nwt = max(nwt, (len(us) + 127) // 128)
    slot_u = np.concatenate(slot_u)
    slot_s = np.concatenate(slot_s)
    slot_q = np.concatenate(slot_q)
    idx_pad = nwt * 128                    # zero window
    assert idx_pad + 127 < 32768

    # --- grid column layout (global K; identical on all devices) ---
    offs = np.concatenate([np.zeros((ngrp, 1), np.int64),
                           np.cumsum(K, axis=1)], axis=1) * (GW * 128)
    Cg = offs[:, -1]
    grpbase = np.concatenate([[0], np.cumsum(Cg)])
    T_dev = int(grpbase[-1])
    assert T_dev % 16 == 0

    # gather split per group: class boundary nearest the middle
    split = []
    for g in range(ngrp):
        sh = int(np.argmin(np.abs(offs[g, 1:-1] - Cg[g] / 2))) + 1
        split.append((sh, int(offs[g, sh])))

    # --- token -> column, idx arrays per device ---
    occ = np.empty(T, np.int64)
    key2 = t_dst * NCLS + cls_tok.astype(np.int64)
    k2o = np.argsort(key2, kind="stable")
    kk2 = key2[k2o]
    bnd = np.concatenate([[True], kk2[1:] != kk2[:-1]])
    gstarts = np.flatnonzero(bnd)
    glens = np.diff(np.concatenate([gstarts, [T]]))
    occ[k2o] = _ragged_arange(glens)

    v = t_dst
    g = grp_n[v]
    col = (grpbase[g] + offs[g, cls_tok.astype(np.int64)]
           + occ * (GW * 128) + (w_n[v] % GW) * 128 + p_n[v])
    tdev = dev_n[v]
    srcval = gidx[t_src, cls_tok.astype(np.int64)].astype(np.int16)
    idxw = np.empty((NDEV, 128, T_dev // 16), np.int16)
    for d in range(NDEV):
        m = tdev == d
        a = np.full(T_dev, idx_pad, np.int16)
        a[col[m]] = srcval[m]
        idxw[d] = np.tile(a.reshape(T_dev // 16, 16).T, (8, 1))

    # --- per-device aux arrays ---
    ridx = np.empty((NDEV, nloc), np.int64)
    for d in range(NDEV):
        gg = ((np.arange(nwin_pad) * NDEV + d)[:, None] * 128 + np.arange(128))
        ridx[d] = gg.reshape(-1)
    node_of_rank = np.full(npad, -1, np.int64)
    node_of_rank[rank_n] = np.arange(N)
    dinv_r = np.zeros(npad, np.float32)
    dinv_r[rank_n] = dinv_n

    dinva = np.empty((NDEV, 128, nwin_pad), np.float32)
    dinvw = np.empty((NDEV, 64, nloc), np.float16)
    for d in range(NDEV):
        dr = dinv_r[ridx[d]]
        dinva[d] = dr.reshape(nwin_pad, 128).T
        dinvw[d] = np.tile(dr[None, :], (64, 1)).astype(np.float16)

    return dict(
        nwin=nwin_pad, ngrp=ngrp, nloc=nloc, npad=npad, nwt=nwt,
        idx_pad=idx_pad, K=K, offs=offs, Cg=Cg, grpbase=grpbase,
        T_dev=T_dev, split=split, idxw=idxw, ridx=ridx,
        node_of_rank=node_of_rank, rank_n=rank_n, dinva=dinva, dinvw=dinvw,
        slot_u=slot_u, slot_s=slot_s, slot_q=slot_q,
    )


# ----------------------------------------------------------------------------
# device programs
# ----------------------------------------------------------------------------

def _build_A(plan):
    nwin, nloc = plan["nwin"], plan["nloc"]
    nc = bacc.Bacc("TRN2", target_bir_lowering=False, debug=False,
                   num_devices=NDEV)
    xT_d = nc.dram_tensor("xT", [DIN, nloc], F16, kind="ExternalInput").ap()
    dinva_d = nc.dram_tensor("dinva", [128, nwin], F32,
                             kind="ExternalInput").ap()
    w1_d = nc.dram_tensor("w1", [128, 2, HID], F16, kind="ExternalInput").ap()
    t1_d = nc.dram_tensor("t1", [nloc, HID], F16, kind="ExternalOutput").ap()

    with tile.TileContext(nc) as tc:
        with (
            tc.tile_pool(name="cst", bufs=1) as cst,
            tc.tile_pool(name="xp", bufs=3) as xp,
            tc.tile_pool(name="ps", bufs=2, space="PSUM") as psp,
            tc.tile_pool(name="stg", bufs=2) as stg,
        ):
            w1t = cst.tile([128, 2, HID], F16)
            nc.sync.dma_start(out=w1t[:], in_=w1_d[:])
            dat = cst.tile([128, nwin], F32)
            nc.sync.dma_start(out=dat[:], in_=dinva_d[:])
            ov = t1_d.rearrange("(w p) f -> p w f", p=128)
            for i0 in range(0, nwin, 8):
                nb = min(8, nwin - i0)
                xts = []
                for k in range(2):
                    xt = xp.tile([128, 8 * 128], F16, tag=f"xt{k}")
                    nc.sync.dma_start(
                        out=xt[:, :nb * 128],
                        in_=xT_d[k * 128:(k + 1) * 128,
                                 i0 * 128:(i0 + nb) * 128],
                    )
                    xts.append(xt)
                stage = stg.tile([128, 8, HID], F16)
                for ib in range(nb):
                    ps = psp.tile([128, HID], F32)
                    for k in range(2):
                        nc.tensor.matmul(
                            out=ps[:],
                            lhsT=xts[k][:, ib * 128:(ib + 1) * 128],
                            rhs=w1t[:, k, :],
                            start=(k == 0), stop=(k == 1),
                        )
                    nc.vector.tensor_scalar_mul(
                        out=stage[:, ib, :], in0=ps[:],
                        scalar1=dat[:, i0 + ib:i0 + ib + 1],
                    )
                nc.sync.dma_start(out=ov[:, i0:i0 + nb, :],
                                  in_=stage[:, :nb, :])
    nc.compile()
    return nc


def _sbuf_gather(nc, out_sl, tabt, idx_sl, ncols, q):
    nc.gpsimd.dma_gather(
        out_sl, tabt[:], idx_sl,
        num_idxs=ncols, num_idxs_reg=ncols, elem_size=128,
        transpose=True, single_packet=False, queue_num=q,
        sbuf_tokens_per_rank=128, sbuf_free_dim_per_rank=256,
        sbuf_free_dim_pad_per_rank=0, sbuf_byte_offset=0,
    )


def _reduce_cls(nc, Ps, vt, c0, Kgs):
    """Full-128-partition strided segment-sum of one class's token columns.
    Only partitions [16s, 16s+16) of the result are meaningful; the selector
    matmul extracts them (DVE cost is free-size driven, partitions are
    parallel lanes, and 16-partition slices at odd 16-offsets violate the
    32-alignment rule)."""
    sl = vt[:, 0, c0:c0 + GW * 128 * Kgs]
    rap = bass.AP(
        sl.tensor, sl.offset,
        [list(sl.ap[0]), [1, GW * 128], [GW * 128, Kgs]],
    )
    nc.vector.tensor_reduce(
        out=Ps[:], in_=rap,
        axis=mybir.AxisListType.X, op=mybir.AluOpType.add,
    )


def _build_BC(plan, layer):
    nwin, ngrp, nloc, nwt = (plan["nwin"], plan["ngrp"], plan["nloc"],
                             plan["nwt"])
    K, offs, Cg, grpbase, split, T_dev = (plan["K"], plan["offs"], plan["Cg"],
                                          plan["grpbase"], plan["split"],
                                          plan["T_dev"])
    WB = GW * 128
    cg_max = max(int(c) for c in Cg)

    nc = bacc.Bacc("TRN2", target_bir_lowering=False, debug=False,
                   num_devices=NDEV, num_swdge_queues=NQUEUES,
                   dynamic_dma_scratch_size=DMASCRATCH)
    tab_d = nc.dram_tensor("tab", [128, (nwt + 1) * 128], F16,
                           kind="ExternalInput").ap()
    idx_d = nc.dram_tensor("idx", [128, T_dev // 16], I16,
                           kind="ExternalInput").ap()
    dinvw_d = nc.dram_tensor("dinvw", [64, nloc], F16,
                             kind="ExternalInput").ap()
    if layer == 1:
        b_d = nc.dram_tensor("b1", [HID, 1], F32, kind="ExternalInput").ap()
        ts_d = nc.dram_tensor("tself", [HID, nloc], F16,
                              kind="ExternalInput").ap()
        sel_d = nc.dram_tensor("sel", [128, NCLS, HID], F32,
                               kind="ExternalInput").ap()
        id_d = nc.dram_tensor("ident", [HID, HID], F16,
                              kind="ExternalInput").ap()
        o_d = nc.dram_tensor("t2", [nloc, HID], F16,
                             kind="ExternalOutput").ap()
        ov = o_d.rearrange("(w p) f -> p w f", p=128)
    else:
        b_d = nc.dram_tensor("b2", [DOUT, 1], F32, kind="ExternalInput").ap()
        ts_d = nc.dram_tensor("wself", [DOUT, nloc], F16,
                              kind="ExternalInput").ap()
        sel_d = nc.dram_tensor("selw2", [128, NCLS, DOUT], F32,
                               kind="ExternalInput").ap()
        o_d = nc.dram_tensor("o2", [DOUT, nloc], F32,
                             kind="ExternalOutput").ap()
        ov = o_d

    with tile.TileContext(nc) as tc:
        with (
            tc.tile_pool(name="cst", bufs=1) as cst,
            tc.tile_pool(name="ip", bufs=3) as ip,
            tc.tile_pool(name="vp", bufs=2) as vp,
            tc.tile_pool(name="pp", bufs=3) as pp,
            tc.tile_pool(name="sm", bufs=3) as sm,
            tc.tile_pool(name="pY", bufs=2, space="PSUM") as pY,
            tc.tile_pool(name="pT", bufs=2, space="PSUM") as pT,
            tc.tile_pool(name="stg", bufs=2) as stg,
        ):
            nc.gpsimd.load_library(library_config.mlp)
            tabt = cst.tile([128, (nwt + 1) * 128], F16)
            nc.sync.dma_start(out=tabt[:], in_=tab_d[:])
            dvw = cst.tile([64, nloc], F16)
            nc.sync.dma_start(out=dvw[:], in_=dinvw_d[:])
            nb_ = HID if layer == 1 else DOUT
            bt = cst.tile([nb_, 1], F32)
            nc.sync.dma_start(out=bt[:], in_=b_d[:])
            tsl = cst.tile([nb_, nloc], F16)
            nc.sync.dma_start(out=tsl[:], in_=ts_d[:])
            selt = cst.tile([128, NCLS, nb_], F32)
            nc.sync.dma_start(out=selt[:], in_=sel_d[:])
            if layer == 1:
                idt = cst.tile([HID, HID], F16)
                nc.sync.dma_start(out=idt[:], in_=id_d[:])

            it_max = cg_max // 16
            qctr = 0
            for g in range(ngrp):
                cg = int(Cg[g])
                t0 = int(grpbase[g])
                it = ip.tile([128, it_max], I16, tag="idx")
                nc.sync.dma_start(out=it[:, :cg // 16],
                                  in_=idx_d[:, t0 // 16:(t0 + cg) // 16])
                va = vp.tile([128, 1, cg_max], F16, tag="va")
                # NOTE: concurrent gathers on rotated SWDGE queues (NQUEUES>1)
                # are 3.5x faster but corrupt data: the per-queue transpose
                # streams interleave packet-wise on the shared XBAR. Keep
                # NQUEUES=1; chunk size amortizes per-instruction overhead
                # within the scratch ring (DMASCRATCH/16 descs per side).
                nch = (cg + GCHUNK - 1) // GCHUNK
                bnds = [(cg * i // nch) // 128 * 128 for i in range(nch + 1)]
                bnds[-1] = cg
                for c0, c1 in zip(bnds[:-1], bnds[1:]):
                    _sbuf_gather(nc, va[:, :, c0:c1], tabt,
                                 it[:, c0 // 16:c1 // 16], c1 - c0,
                                 qctr % NQUEUES)
                    qctr += 1
                Y = pY.tile([nb_, WB], F32)
                for s in range(NCLS):
                    Kgs = int(K[g, s])
                    Ps = pp.tile([128, WB], F32, tag=f"P{s}")
                    _reduce_cls(nc, Ps, va, int(offs[g, s]), Kgs)
                    nc.tensor.matmul(out=Y[:], lhsT=selt[:, s, :], rhs=Ps[:],
                                     start=(s == 0), stop=(s == NCLS - 1))
                dsl = dvw[0:nb_, g * WB:(g + 1) * WB]
                ya = sm.tile([nb_, WB], F32, tag="ya")
                nc.vector.scalar_tensor_tensor(
                    out=ya[:], in0=Y[:], scalar=1.0,
                    in1=tsl[:, g * WB:(g + 1) * WB],
                    op0=mybir.AluOpType.mult, op1=mybir.AluOpType.add,
                )
                if layer == 1:
                    yd = sm.tile([HID, WB], F32, tag="yd")
                    nc.vector.scalar_tensor_tensor(
                        out=yd[:], in0=ya[:], scalar=1.0, in1=dsl,
                        op0=mybir.AluOpType.mult, op1=mybir.AluOpType.mult,
                    )
                    r = sm.tile([HID, WB], F32, tag="r")
                    nc.scalar.activation(
                        out=r[:], in_=yd[:],
                        func=mybir.ActivationFunctionType.Relu,
                        bias=bt[:, 0:1],
                    )
                    t2c = sm.tile([HID, WB], F16, tag="t2c")
                    nc.vector.scalar_tensor_tensor(
                        out=t2c[:], in0=r[:], scalar=1.0, in1=dsl,
                        op0=mybir.AluOpType.mult, op1=mybir.AluOpType.mult,
                    )
                    stage = stg.tile([128, GW, HID], F16)
                    for j in range(GW):
                        tp = pT.tile([128, HID], F16)
                        nc.tensor.transpose(
                            out=tp[:], in_=t2c[:, j * 128:(j + 1) * 128],
                            identity=idt[:])
                        nc.vector.tensor_copy(out=stage[:, j, :], in_=tp[:])
                    nc.sync.dma_start(out=ov[:, g * GW:(g + 1) * GW, :],
                                      in_=stage[:])
                else:
                    od = sm.tile([DOUT, WB], F32, tag="od")
                    nc.vector.scalar_tensor_tensor(
                        out=od[:], in0=ya[:], scalar=1.0, in1=dsl,
                        op0=mybir.AluOpType.mult, op1=mybir.AluOpType.mult,
                    )
                    stage = stg.tile([DOUT, WB], F32)
                    nc.vector.tensor_scalar_add(
                        out=stage[:], in0=od[:], scalar1=bt[:, 0:1],
                    )
                    nc.sync.dma_start(out=ov[:, g * WB:(g + 1) * WB],
                                      in_=stage[:])
    nc.compile()
    return nc


# ----------------------------------------------------------------------------
# driver
# ----------------------------------------------------------------------------

_PROG_CACHE = {}


def _run(nc, in_maps):
    trace = os.environ.get("GCN_TRACE", "0") == "1"
    res = bass_utils.run_bass_kernel_spmd(
        nc, in_maps, core_ids=list(range(NDEV)), trace=trace)
    if res.exec_time_ns is not None:
        LAST_EXEC_NS.append(int(res.exec_time_ns))
    return res.results


def _build_table(plan, t_node):
    nwt = plan["nwt"]
    tab = np.zeros((128, nwt + 1, NCLS, HID), np.float16)
    q, s, u = plan["slot_q"], plan["slot_s"], plan["slot_u"]
    tab[q % 128, q // 128, s] = t_node[u]
    return np.ascontiguousarray(tab.reshape(128, (nwt + 1) * 128))


def kernel(x, edge_index, W1, b1, W2, b2):
    LAST_EXEC_NS.clear()
    x = np.asarray(x, np.float32)
    W1 = np.asarray(W1, np.float32)
    b1 = np.asarray(b1, np.float32)
    W2 = np.asarray(W2, np.float32)
    b2 = np.asarray(b2, np.float32)

    ei = np.asarray(edge_index)
    plan = _plan(ei, seed=99)
    key = ("v6", plan["nwin"], plan["T_dev"], int(plan["K"].sum()))
    if key not in _PROG_CACHE:
        _PROG_CACHE.clear()
        _PROG_CACHE[key] = (_build_A(plan), _build_BC(plan, 1),
                            _build_BC(plan, 2))
    ncA, ncB, ncC = _PROG_CACHE[key]

    ridx, nor = plan["ridx"], plan["node_of_rank"]
    npad, nloc = plan["npad"], plan["nloc"]

    xfull = np.zeros((npad, DIN), np.float32)
    xfull[plan["rank_n"]] = x
    w1r = np.ascontiguousarray(
        W1.reshape(2, 128, HID).transpose(1, 0, 2)).astype(np.float16)
    inA = [{"xT": np.ascontiguousarray(xfull[ridx[d]].T).astype(np.float16),
            "dinva": plan["dinva"][d], "w1": w1r} for d in range(NDEV)]
    resA = _run(ncA, inA)
    t1n = np.zeros((N, HID), np.float16)
    for d in range(NDEV):
        m = nor[ridx[d]] >= 0
        t1n[nor[ridx[d]][m]] = resA[d]["t1"][m]

    sel = np.zeros((128, NCLS, HID), np.float32)
    for s in range(NCLS):
        sel[16 * s + np.arange(HID), s, np.arange(HID)] = 1.0
    def _self_arr(tn):
        out = []
        for d in range(NDEV):
            a = np.zeros((nloc, tn.shape[1]), np.float16)
            m = nor[ridx[d]] >= 0
            a[m] = tn[nor[ridx[d]][m]]
            out.append(np.ascontiguousarray(a.T))
        return out

    ts1 = _self_arr(t1n)
    inB = [{"tab": _build_table(plan, t1n), "idx": plan["idxw"][d],
            "dinvw": plan["dinvw"][d], "b1": b1[:, None].astype(np.float32),
            "tself": ts1[d], "sel": sel,
            "ident": np.eye(HID, dtype=np.float16)}
           for d in range(NDEV)]
    resB = _run(ncB, inB)
    t2n = np.zeros((N, HID), np.float16)
    for d in range(NDEV):
        m = nor[ridx[d]] >= 0
        t2n[nor[ridx[d]][m]] = resB[d]["t2"][m]

    selw2 = np.zeros((128, NCLS, DOUT), np.float32)
    for s in range(NCLS):
        selw2[16 * s + np.arange(HID), s, :] = W2
    ws = _self_arr((t2n.astype(np.float32) @ W2).astype(np.float16))
    inC = [{"tab": _build_table(plan, t2n), "idx": plan["idxw"][d],
            "dinvw": plan["dinvw"][d], "b2": b2[:, None].astype(np.float32),
            "wself": ws[d], "selw2": selw2} for d in range(NDEV)]
    resC = _run(ncC, inC)
    out = np.zeros((N, DOUT), np.float32)
    for d in range(NDEV):
        m = nor[ridx[d]] >= 0
        out[nor[ridx[d]][m]] = resC[d]["o2"].T[m]
    return out



# revision 16
# speedup vs baseline: 1.2171x; 1.0031x over previous
"""Trainium2 Bass kernel for 2-layer GCN (GCNConv -> ReLU -> GCNConv).

v2 strategy — SBUF-resident fp16 tables + transpose-mode SBUF-source gathers
(the baseline's HBM dma_gather was HBM-latency bound at ~63ns/edge):

- Both layers reduce to: gather 16-wide rows t[src], segment-sum by dst
  (linear layers commute with the normalized aggregation).
- The 16-fp16 (32B) node records live in SBUF as [128 part, W windows] of
  256B chunks; chunk (tok, w) holds 8 records at positions s=0..7.
- A token (edge) gathers its source's 256B chunk via dma_gather(transpose=
  True, SBUF source): the chunk becomes a 128-partition fp16 column; the
  wanted record sits at partition slice [16s, 16s+16) where s = the record's
  chunk position ("class").  Chunk-mates land on other slices — never read.
- Host assigns each node TWO candidate classes and each edge picks one
  (power-of-two-choices), balancing per-(destination-group, class) slot
  counts K.  Grid columns per group g: [class s][slot k<K[g,s]][win j][dst p]
  so one strided DVE tensor_reduce per (group, class) segment-sums slot
  layers into slice s of a [128, 256] tile P.  P's 8 slices collapse via a
  PE matmul with a 0/1 selector (layer 2 fuses W2 into the selector).
- 3 SPMD launches: A (t1 = dinv*x@W1), B (layer-1 aggregate -> t2),
  C (layer-2 aggregate -> @W2+b2).  Host re-shards tables between launches.
"""
import os
import sys

sys.path.insert(0, "/opt/trn_rl_repo")

import numpy as np

import concourse.bass as bass
import concourse.mybir as mybir
import concourse.tile as tile
from concourse import bacc, bass_utils, library_config

N = 100000
E = 1600000
DIN, HID, DOUT = 256, 16, 64
NDEV = 8
NCLS = 8
GW = 1                      # windows per K-uniform group
F32 = mybir.dt.float32
F16 = mybir.dt.float16
I16 = mybir.dt.int16
NQUEUES = int(os.environ.get("GCN_NQUEUES", "1"))
GCHUNK = int(os.environ.get("GCN_GCHUNK", "2944"))
DMASCRATCH = int(os.environ.get("GCN_DMASCRATCH", "49152"))
N3 = int(os.environ.get("GCN_N3", "62000"))
MAXSLOTS = 32640              # 255 windows/class keeps idx_pad in int16

LAST_EXEC_NS = []


# ----------------------------------------------------------------------------
# host-side graph planning
# ----------------------------------------------------------------------------

def _ragged_arange(lens):
    ends = np.cumsum(lens)
    total = int(ends[-1]) if len(lens) else 0
    out = np.arange(total, dtype=np.int64)
    out -= np.repeat(ends - lens, lens)
    return out


def _plan(edge_index, seed=12345):
    rng = np.random.default_rng(seed)
    src = np.asarray(edge_index[0], dtype=np.int64)
    dst = np.asarray(edge_index[1], dtype=np.int64)
    # self-loops are NOT tokens: the dst's own record is added in postproc
    all_src = src
    all_dst = dst
    T = len(all_src)
    indeg = np.bincount(dst, minlength=N).astype(np.int64) + 1  # GCN degree
    dinv_n = (1.0 / np.sqrt(indeg.astype(np.float64))).astype(np.float32)

    # rank deal: degree-sorted; i-th -> device i%8, window (i//8)//128
    order = np.argsort(-indeg, kind="stable")
    di = np.empty(N, np.int64)
    di[order] = np.arange(N)
    dev_n = di % NDEV
    w_n = (di // NDEV) // 128
    p_n = (di // NDEV) % 128
    nwin = int(w_n.max()) + 1
    ngrp = (nwin + GW - 1) // GW
    nwin_pad = ngrp * GW
    rank_n = (w_n * NDEV + dev_n) * 128 + p_n
    nloc = nwin_pad * 128
    npad = nloc * NDEV

    grp_n = w_n // GW

    # --- class assignment: mixed-R candidates + balanced greedy + repair ---
    ko = np.argsort(all_dst, kind="stable")
    t_dst = all_dst[ko]
    t_src = all_src[ko]
    tg = grp_n[t_dst]

    def _make_cand(n3):
        s1 = rng.integers(0, NCLS, N)
        s2 = (s1 + 1 + rng.integers(0, NCLS - 1, N)) % NCLS
        cand = np.stack([s1, s2], 1)
        if n3 > 0:
            odeg = np.bincount(all_src, minlength=N)
            top = np.argsort(-odeg, kind="stable")[:n3]
            s3 = rng.integers(0, NCLS, n3)
            bad = (s3 == s1[top]) | (s3 == s2[top])
            while bad.any():
                s3[bad] = rng.integers(0, NCLS, bad.sum())
                bad = (s3 == s1[top]) | (s3 == s2[top])
            c3 = np.full(N, -1, np.int64)
            c3[top] = s3
            cand = np.concatenate([cand, c3[:, None]], 1)
        return cand

    starts = np.searchsorted(t_dst, np.arange(N + 1))
    pos = np.arange(T) - np.repeat(starts[:-1], np.diff(starts))

    def _greedy(cand):
        cnt = np.zeros((N, NCLS), np.int16)
        cls_tok = np.zeros(T, np.int8)
        for k in range(int(pos.max()) + 1):
            m = np.flatnonzero(pos == k)
            if len(m) == 0:
                break
            u, v = t_src[m], t_dst[m]
            cc = cand[u]
            counts = np.where(cc >= 0, cnt[v[:, None], np.maximum(cc, 0)], 127)
            best = np.argmin(counts, axis=1)
            c = cc[np.arange(len(m)), best]
            cls_tok[m] = c
            cnt[v, c] += 1
        return cls_tok, cnt

    def _repair(cls_tok, cnt, cand, iters=400):
        R = cand.shape[1]
        cls_tok = cls_tok.astype(np.int64)
        tc = cand[t_src]
        for it in range(iters):
            K = np.zeros((ngrp, NCLS), np.int64)
            np.maximum.at(K, (tg, cls_tok), cnt[t_dst, cls_tok])
            K = np.maximum(K, 1)
            cur = cls_tok
            crit = cnt[t_dst, cur] == K[tg, cur]
            alt_ok = np.full(T, -1, np.int64)
            for r in rng.permutation(R):
                a = tc[:, r]
                a0 = np.maximum(a, 0)
                ok = (crit & (a >= 0) & (a != cur) & (alt_ok < 0)
                      & (cnt[t_dst, a0] + 1 < K[tg, a0]))
                alt_ok[ok] = a[ok]
            mv = np.flatnonzero(alt_ok >= 0)
            if len(mv) == 0:
                # 2-chain: evict a non-critical blocker from (dst, a) where a
                # sits one below the group max, freeing room for a critical
                # token to move there next iteration.
                relief = np.full(T, -1, np.int64)
                for r in rng.permutation(R):
                    a = tc[:, r]
                    a0 = np.maximum(a, 0)
                    ok = (crit & (a >= 0) & (a != cur) & (relief < 0)
                          & (cnt[t_dst, a0] + 1 == K[tg, a0])
                          & (cnt[t_dst, a0] > 0))
                    relief[ok] = a[ok]
                want = np.flatnonzero(relief >= 0)
                if len(want) == 0:
                    break
                wantset = np.zeros(N * NCLS, bool)
                wantset[t_dst[want] * NCLS + relief[want]] = True
                isblk = wantset[t_dst * NCLS + cur] & ~crit
                blk_alt = np.full(T, -1, np.int64)
                for r in rng.permutation(R):
                    a = tc[:, r]
                    a0 = np.maximum(a, 0)
                    ok = (isblk & (a >= 0) & (a != cur) & (blk_alt < 0)
                          & (cnt[t_dst, a0] + 1 < K[tg, a0]))
                    blk_alt[ok] = a[ok]
                mv = np.flatnonzero(blk_alt >= 0)
                if len(mv) == 0:
                    break
                alt_ok = blk_alt
            ks = np.argsort(t_dst[mv], kind="stable")
            kk = t_dst[mv][ks]
            first = np.concatenate([[True], kk[1:] != kk[:-1]])
            mv = mv[ks[first]]
            cv, av, vv = cur[mv].copy(), alt_ok[mv], t_dst[mv]
            cls_tok[mv] = av
            np.subtract.at(cnt, (vv, cv), 1)
            np.add.at(cnt, (vv, av), 1)
        K = np.zeros((ngrp, NCLS), np.int64)
        np.maximum.at(K, (tg, cls_tok), cnt[t_dst, cls_tok])
        return cls_tok.astype(np.int8), cnt, np.maximum(K, 1)

    n3 = N3
    while True:
        cand = _make_cand(n3)
        cls_tok, cnt = _greedy(cand)
        cls_tok, cnt, K = _repair(cls_tok, cnt, cand)
        used_chk = np.zeros((N, NCLS), bool)
        used_chk[t_src, cls_tok.astype(np.int64)] = True
        if used_chk.sum(0).max() <= MAXSLOTS or n3 == 0:
            break
        n3 = max(0, n3 - 15000)

    # --- table slot allocation per class ---
    used = np.zeros((N, NCLS), bool)
    used[t_src, cls_tok.astype(np.int64)] = True
    gidx = np.zeros((N, NCLS), np.int32)
    slot_u, slot_s, slot_q = [], [], []
    nwt = 0
    for s in range(NCLS):
        us = np.flatnonzero(used[:, s])
        q = np.arange(len(us))
        gidx[us, s] = (q // 128) * 128 + (q % 128)
        slot_u.append(us)
        slot_s.append(np.full(len(us), s))
        slot_q.append(q)
        nwt = max(nwt, (len(us) + 127) // 128)
    slot_u = np.concatenate(slot_u)
    slot_s = np.concatenate(slot_s)
    slot_q = np.concatenate(slot_q)
    idx_pad = nwt * 128                    # zero window
    assert idx_pad + 127 < 32768

    # --- grid column layout (global K; identical on all devices) ---
    offs = np.concatenate([np.zeros((ngrp, 1), np.int64),
                           np.cumsum(K, axis=1)], axis=1) * (GW * 128)
    Cg = offs[:, -1]
    grpbase = np.concatenate([[0], np.cumsum(Cg)])
    T_dev = int(grpbase[-1])
    assert T_dev % 16 == 0

    # gather split per group: class boundary nearest the middle
    split = []
    for g in range(ngrp):
        sh = int(np.argmin(np.abs(offs[g, 1:-1] - Cg[g] / 2))) + 1
        split.append((sh, int(offs[g, sh])))

    # --- token -> column, idx arrays per device ---
    occ = np.empty(T, np.int64)
    key2 = t_dst * NCLS + cls_tok.astype(np.int64)
    k2o = np.argsort(key2, kind="stable")
    kk2 = key2[k2o]
    bnd = np.concatenate([[True], kk2[1:] != kk2[:-1]])
    gstarts = np.flatnonzero(bnd)
    glens = np.diff(np.concatenate([gstarts, [T]]))
    occ[k2o] = _ragged_arange(glens)

    v = t_dst
    g = grp_n[v]
    col = (grpbase[g] + offs[g, cls_tok.astype(np.int64)]
           + occ * (GW * 128) + (w_n[v] % GW) * 128 + p_n[v])
    tdev = dev_n[v]
    srcval = gidx[t_src, cls_tok.astype(np.int64)].astype(np.int16)
    idxw = np.empty((NDEV, 128, T_dev // 16), np.int16)
    for d in range(NDEV):
        m = tdev == d
        a = np.full(T_dev, idx_pad, np.int16)
        a[col[m]] = srcval[m]
        idxw[d] = np.tile(a.reshape(T_dev // 16, 16).T, (8, 1))

    # --- per-device aux arrays ---
    ridx = np.empty((NDEV, nloc), np.int64)
    for d in range(NDEV):
        gg = ((np.arange(nwin_pad) * NDEV + d)[:, None] * 128 + np.arange(128))
        ridx[d] = gg.reshape(-1)
    node_of_rank = np.full(npad, -1, np.int64)
    node_of_rank[rank_n] = np.arange(N)
    dinv_r = np.zeros(npad, np.float32)
    dinv_r[rank_n] = dinv_n

    dinva = np.empty((NDEV, 128, nwin_pad), np.float32)
    dinvw = np.empty((NDEV, 64, nloc), np.float16)
    for d in range(NDEV):
        dr = dinv_r[ridx[d]]
        dinva[d] = dr.reshape(nwin_pad, 128).T
        dinvw[d] = np.tile(dr[None, :], (64, 1)).astype(np.float16)

    return dict(
        nwin=nwin_pad, ngrp=ngrp, nloc=nloc, npad=npad, nwt=nwt,
        idx_pad=idx_pad, K=K, offs=offs, Cg=Cg, grpbase=grpbase,
        T_dev=T_dev, split=split, idxw=idxw, ridx=ridx,
        node_of_rank=node_of_rank, rank_n=rank_n, dinva=dinva, dinvw=dinvw,
        slot_u=slot_u, slot_s=slot_s, slot_q=slot_q,
    )


# ----------------------------------------------------------------------------
# device programs
# ----------------------------------------------------------------------------

def _build_A(plan):
    nwin, nloc = plan["nwin"], plan["nloc"]
    nc = bacc.Bacc("TRN2", target_bir_lowering=False, debug=False,
                   num_devices=NDEV)
    xT_d = nc.dram_tensor("xT", [DIN, nloc], F16, kind="ExternalInput").ap()
    dinva_d = nc.dram_tensor("dinva", [128, nwin], F32,
                             kind="ExternalInput").ap()
    w1_d = nc.dram_tensor("w1", [128, 2, HID], F16, kind="ExternalInput").ap()
    t1_d = nc.dram_tensor("t1", [nloc, HID], F16, kind="ExternalOutput").ap()

    with tile.TileContext(nc) as tc:
        with (
            tc.tile_pool(name="cst", bufs=1) as cst,
            tc.tile_pool(name="xp", bufs=3) as xp,
            tc.tile_pool(name="ps", bufs=2, space="PSUM") as psp,
            tc.tile_pool(name="stg", bufs=2) as stg,
        ):
            w1t = cst.tile([128, 2, HID], F16)
            nc.sync.dma_start(out=w1t[:], in_=w1_d[:])
            dat = cst.tile([128, nwin], F32)
            nc.sync.dma_start(out=dat[:], in_=dinva_d[:])
            ov = t1_d.rearrange("(w p) f -> p w f", p=128)
            for i0 in range(0, nwin, 8):
                nb = min(8, nwin - i0)
                xts = []
                for k in range(2):
                    xt = xp.tile([128, 8 * 128], F16, tag=f"xt{k}")
                    nc.sync.dma_start(
                        out=xt[:, :nb * 128],
                        in_=xT_d[k * 128:(k + 1) * 128,
                                 i0 * 128:(i0 + nb) * 128],
                    )
                    xts.append(xt)
                stage = stg.tile([128, 8, HID], F16)
                for ib in range(nb):
                    ps = psp.tile([128, HID], F32)
                    for k in range(2):
                        nc.tensor.matmul(
                            out=ps[:],
                            lhsT=xts[k][:, ib * 128:(ib + 1) * 128],
                            rhs=w1t[:, k, :],
                            start=(k == 0), stop=(k == 1),
                        )
                    nc.vector.tensor_scalar_mul(
                        out=stage[:, ib, :], in0=ps[:],
                        scalar1=dat[:, i0 + ib:i0 + ib + 1],
                    )
                nc.sync.dma_start(out=ov[:, i0:i0 + nb, :],
                                  in_=stage[:, :nb, :])
    nc.compile()
    return nc


def _sbuf_gather(nc, out_sl, tabt, idx_sl, ncols, q):
    nc.gpsimd.dma_gather(
        out_sl, tabt[:], idx_sl,
        num_idxs=ncols, num_idxs_reg=ncols, elem_size=128,
        transpose=True, single_packet=False, queue_num=q,
        sbuf_tokens_per_rank=128, sbuf_free_dim_per_rank=256,
        sbuf_free_dim_pad_per_rank=0, sbuf_byte_offset=0,
    )


def _reduce_cls(nc, Ps, vt, c0, Kgs):
    """Full-128-partition strided segment-sum of one class's token columns.
    Only partitions [16s, 16s+16) of the result are meaningful; the selector
    matmul extracts them (DVE cost is free-size driven, partitions are
    parallel lanes, and 16-partition slices at odd 16-offsets violate the
    32-alignment rule)."""
    sl = vt[:, 0, c0:c0 + GW * 128 * Kgs]
    rap = bass.AP(
        sl.tensor, sl.offset,
        [list(sl.ap[0]), [1, GW * 128], [GW * 128, Kgs]],
    )
    nc.vector.tensor_reduce(
        out=Ps[:], in_=rap,
        axis=mybir.AxisListType.X, op=mybir.AluOpType.add,
    )


def _build_BC(plan, layer):
    nwin, ngrp, nloc, nwt = (plan["nwin"], plan["ngrp"], plan["nloc"],
                             plan["nwt"])
    K, offs, Cg, grpbase, split, T_dev = (plan["K"], plan["offs"], plan["Cg"],
                                          plan["grpbase"], plan["split"],
                                          plan["T_dev"])
    WB = GW * 128
    cg_max = max(int(c) for c in Cg)

    nc = bacc.Bacc("TRN2", target_bir_lowering=False, debug=False,
                   num_devices=NDEV, num_swdge_queues=NQUEUES,
                   dynamic_dma_scratch_size=DMASCRATCH)
    tab_d = nc.dram_tensor("tab", [128, (nwt + 1) * 128], F16,
                           kind="ExternalInput").ap()
    idx_d = nc.dram_tensor("idx", [128, T_dev // 16], I16,
                           kind="ExternalInput").ap()
    dinvw_d = nc.dram_tensor("dinvw", [64, nloc], F16,
                             kind="ExternalInput").ap()
    if layer == 1:
        b_d = nc.dram_tensor("b1", [HID, 1], F32, kind="ExternalInput").ap()
        ts_d = nc.dram_tensor("tself", [HID, nloc], F16,
                              kind="ExternalInput").ap()
        sel_d = nc.dram_tensor("sel", [128, NCLS, HID], F32,
                               kind="ExternalInput").ap()
        id_d = nc.dram_tensor("ident", [HID, HID], F16,
                              kind="ExternalInput").ap()
        o_d = nc.dram_tensor("t2", [nloc, HID], F16,
                             kind="ExternalOutput").ap()
        ov = o_d.rearrange("(w p) f -> p w f", p=128)
    else:
        b_d = nc.dram_tensor("b2", [DOUT, 1], F32, kind="ExternalInput").ap()
        ts_d = nc.dram_tensor("wself", [DOUT, nloc], F16,
                              kind="ExternalInput").ap()
        sel_d = nc.dram_tensor("selw2", [128, NCLS, DOUT], F32,
                               kind="ExternalInput").ap()
        o_d = nc.dram_tensor("o2", [DOUT, nloc], F16,
                             kind="ExternalOutput").ap()
        ov = o_d

    with tile.TileContext(nc) as tc:
        with (
            tc.tile_pool(name="cst", bufs=1) as cst,
            tc.tile_pool(name="ip", bufs=3) as ip,
            tc.tile_pool(name="vp", bufs=2) as vp,
            tc.tile_pool(name="pp", bufs=3) as pp,
            tc.tile_pool(name="sm", bufs=3) as sm,
            tc.tile_pool(name="pY", bufs=2, space="PSUM") as pY,
            tc.tile_pool(name="pT", bufs=2, space="PSUM") as pT,
            tc.tile_pool(name="stg", bufs=2) as stg,
        ):
            nc.gpsimd.load_library(library_config.mlp)
            tabt = cst.tile([128, (nwt + 1) * 128], F16)
            nc.sync.dma_start(out=tabt[:], in_=tab_d[:])
            dvw = cst.tile([64, nloc], F16)
            nc.scalar.dma_start(out=dvw[:], in_=dinvw_d[:])
            nb_ = HID if layer == 1 else DOUT
            bt = cst.tile([nb_, 1], F32)
            nc.scalar.dma_start(out=bt[:], in_=b_d[:])
            tsl = cst.tile([nb_, nloc], F16)
            nc.scalar.dma_start(out=tsl[:], in_=ts_d[:])
            selt = cst.tile([128, NCLS, nb_], F32)
            nc.scalar.dma_start(out=selt[:], in_=sel_d[:])
            if layer == 1:
                idt = cst.tile([HID, HID], F16)
                nc.scalar.dma_start(out=idt[:], in_=id_d[:])

            it_max = cg_max // 16
            qctr = 0
            for g in range(ngrp):
                cg = int(Cg[g])
                t0 = int(grpbase[g])
                it = ip.tile([128, it_max], I16, tag="idx")
                nc.sync.dma_start(out=it[:, :cg // 16],
                                  in_=idx_d[:, t0 // 16:(t0 + cg) // 16])
                va = vp.tile([128, 1, cg_max], F16, tag="va")
                # NOTE: concurrent gathers on rotated SWDGE queues (NQUEUES>1)
                # are 3.5x faster but corrupt data: the per-queue transpose
                # streams interleave packet-wise on the shared XBAR. Keep
                # NQUEUES=1; chunk size amortizes per-instruction overhead
                # within the scratch ring (DMASCRATCH/16 descs per side).
                nch = (cg + GCHUNK - 1) // GCHUNK
                bnds = [(cg * i // nch) // 128 * 128 for i in range(nch + 1)]
                bnds[-1] = cg
                for c0, c1 in zip(bnds[:-1], bnds[1:]):
                    _sbuf_gather(nc, va[:, :, c0:c1], tabt,
                                 it[:, c0 // 16:c1 // 16], c1 - c0,
                                 qctr % NQUEUES)
                    qctr += 1
                Y = pY.tile([nb_, WB], F32)
                for s in range(NCLS):
                    Kgs = int(K[g, s])
                    Ps = pp.tile([128, WB], F32, tag=f"P{s}")
                    _reduce_cls(nc, Ps, va, int(offs[g, s]), Kgs)
                    nc.tensor.matmul(out=Y[:], lhsT=selt[:, s, :], rhs=Ps[:],
                                     start=(s == 0), stop=(s == NCLS - 1))
                dsl = dvw[0:nb_, g * WB:(g + 1) * WB]
                ya = sm.tile([nb_, WB], F32, tag="ya")
                nc.vector.scalar_tensor_tensor(
                    out=ya[:], in0=Y[:], scalar=1.0,
                    in1=tsl[:, g * WB:(g + 1) * WB],
                    op0=mybir.AluOpType.mult, op1=mybir.AluOpType.add,
                )
                if layer == 1:
                    yd = sm.tile([HID, WB], F32, tag="yd")
                    nc.vector.scalar_tensor_tensor(
                        out=yd[:], in0=ya[:], scalar=1.0, in1=dsl,
                        op0=mybir.AluOpType.mult, op1=mybir.AluOpType.mult,
                    )
                    r = sm.tile([HID, WB], F32, tag="r")
                    nc.scalar.activation(
                        out=r[:], in_=yd[:],
                        func=mybir.ActivationFunctionType.Relu,
                        bias=bt[:, 0:1],
                    )
                    t2c = sm.tile([HID, WB], F16, tag="t2c")
                    nc.vector.scalar_tensor_tensor(
                        out=t2c[:], in0=r[:], scalar=1.0, in1=dsl,
                        op0=mybir.AluOpType.mult, op1=mybir.AluOpType.mult,
                    )
                    stage = stg.tile([128, GW, HID], F16)
                    for j in range(GW):
                        tp = pT.tile([128, HID], F16)
                        nc.tensor.transpose(
                            out=tp[:], in_=t2c[:, j * 128:(j + 1) * 128],
                            identity=idt[:])
                        nc.vector.tensor_copy(out=stage[:, j, :], in_=tp[:])
                    nc.sync.dma_start(out=ov[:, g * GW:(g + 1) * GW, :],
                                      in_=stage[:])
                else:
                    od = sm.tile([DOUT, WB], F32, tag="od")
                    nc.vector.scalar_tensor_tensor(
                        out=od[:], in0=ya[:], scalar=1.0, in1=dsl,
                        op0=mybir.AluOpType.mult, op1=mybir.AluOpType.mult,
                    )
                    stage = stg.tile([DOUT, WB], F16)
                    nc.vector.tensor_scalar_add(
                        out=stage[:], in0=od[:], scalar1=bt[:, 0:1],
                    )
                    nc.sync.dma_start(out=ov[:, g * WB:(g + 1) * WB],
                                      in_=stage[:])
    nc.compile()
    return nc


# ----------------------------------------------------------------------------
# driver
# ----------------------------------------------------------------------------

_PROG_CACHE = {}


def _run(nc, in_maps):
    trace = os.environ.get("GCN_TRACE", "0") == "1"
    res = bass_utils.run_bass_kernel_spmd(
        nc, in_maps, core_ids=list(range(NDEV)), trace=trace)
    if res.exec_time_ns is not None:
        LAST_EXEC_NS.append(int(res.exec_time_ns))
    return res.results


def _build_table(plan, t_node):
    nwt = plan["nwt"]
    tab = np.zeros((128, nwt + 1, NCLS, HID), np.float16)
    q, s, u = plan["slot_q"], plan["slot_s"], plan["slot_u"]
    tab[q % 128, q // 128, s] = t_node[u]
    return np.ascontiguousarray(tab.reshape(128, (nwt + 1) * 128))


def kernel(x, edge_index, W1, b1, W2, b2):
    LAST_EXEC_NS.clear()
    x = np.asarray(x, np.float32)
    W1 = np.asarray(W1, np.float32)
    b1 = np.asarray(b1, np.float32)
    W2 = np.asarray(W2, np.float32)
    b2 = np.asarray(b2, np.float32)

    ei = np.asarray(edge_index)
    plan = _plan(ei, seed=99)
    key = ("v6", plan["nwin"], plan["T_dev"], int(plan["K"].sum()))
    if key not in _PROG_CACHE:
        _PROG_CACHE.clear()
        _PROG_CACHE[key] = (_build_A(plan), _build_BC(plan, 1),
                            _build_BC(plan, 2))
    ncA, ncB, ncC = _PROG_CACHE[key]

    ridx, nor = plan["ridx"], plan["node_of_rank"]
    npad, nloc = plan["npad"], plan["nloc"]

    xfull = np.zeros((npad, DIN), np.float32)
    xfull[plan["rank_n"]] = x
    w1r = np.ascontiguousarray(
        W1.reshape(2, 128, HID).transpose(1, 0, 2)).astype(np.float16)
    inA = [{"xT": np.ascontiguousarray(xfull[ridx[d]].T).astype(np.float16),
            "dinva": plan["dinva"][d], "w1": w1r} for d in range(NDEV)]
    resA = _run(ncA, inA)
    t1n = np.zeros((N, HID), np.float16)
    for d in range(NDEV):
        m = nor[ridx[d]] >= 0
        t1n[nor[ridx[d]][m]] = resA[d]["t1"][m]

    sel = np.zeros((128, NCLS, HID), np.float32)
    for s in range(NCLS):
        sel[16 * s + np.arange(HID), s, np.arange(HID)] = 1.0
    def _self_arr(tn):
        out = []
        for d in range(NDEV):
            a = np.zeros((nloc, tn.shape[1]), np.float16)
            m = nor[ridx[d]] >= 0
            a[m] = tn[nor[ridx[d]][m]]
            out.append(np.ascontiguousarray(a.T))
        return out

    ts1 = _self_arr(t1n)
    inB = [{"tab": _build_table(plan, t1n), "idx": plan["idxw"][d],
            "dinvw": plan["dinvw"][d], "b1": b1[:, None].astype(np.float32),
            "tself": ts1[d], "sel": sel,
            "ident": np.eye(HID, dtype=np.float16)}
           for d in range(NDEV)]
    resB = _run(ncB, inB)
    t2n = np.zeros((N, HID), np.float16)
    for d in range(NDEV):
        m = nor[ridx[d]] >= 0
        t2n[nor[ridx[d]][m]] = resB[d]["t2"][m]

    selw2 = np.zeros((128, NCLS, DOUT), np.float32)
    for s in range(NCLS):
        selw2[16 * s + np.arange(HID), s, :] = W2
    ws = _self_arr((t2n.astype(np.float32) @ W2).astype(np.float16))
    inC = [{"tab": _build_table(plan, t2n), "idx": plan["idxw"][d],
            "dinvw": plan["dinvw"][d], "b2": b2[:, None].astype(np.float32),
            "wself": ws[d], "selw2": selw2} for d in range(NDEV)]
    resC = _run(ncC, inC)
    out = np.zeros((N, DOUT), np.float32)
    for d in range(NDEV):
        m = nor[ridx[d]] >= 0
        out[nor[ridx[d]][m]] = resC[d]["o2"].T[m]
    return out

